# revision 1
# baseline (speedup 1.0000x reference)
"""MetaConvSmoother Trainium2 kernel (Bass/Tile), data-parallel over 8 NeuronCores.

Per core (8 samples):
  - hypernet MLPs (9 -> 100 -> 147, exact gelu) on PE + ACT
  - per-sample conv kernels staged as zero-padded tables in DRAM
    (one 255-float table per (sample, tap-column); U[127-ky] = w[ky, kx])
  - flipped Toeplitz bands Bf[i, m] = U[i+m] loaded with all-positive strided
    DMAs, then partition-reversed on the TensorEngine with a constant
    anti-diagonal matrix: B[p, m] = U[127 + m - p]
  - each conv stage = banded matmuls over image rows (lhsT = B slices),
    column taps via free-dim offset reads of the rhs tile, PSUM accumulation:
      Ax   : 3x3, asymmetric pad (top/left 0, bottom/right 1.0)
      tmp_m: 7x7 corr of r = f - Ax        (3 maps)
      G2   : sum_m 7x7 corr of tmp_m
      out  = x + G2
  - r and tmp round-trip through DRAM to decouple row-tile alignments
  - DMA engine split to avoid FIFO head-of-line blocking:
      SP   : independent loads (x, f, bands)
      POOL : dependent loads (r-in, tmp-in) + table scatter writes (SWDGE)
      ACT  : stores (r-out, tmp-out, out) + band-reversal PSUM->SBUF copies
      DVE  : sub/add/copies/memsets
"""
import numpy as np

import concourse.bass as bass
import concourse.mybir as mybir
from concourse import bacc, bass_utils
from concourse.tile import TileContext

F32 = mybir.dt.float32
F32R = mybir.dt.float32r
USE_F32R = True


def _cast(ap):
    return ap
S = 8          # samples per core
N = 512
ML = 3
KK = 7
NCORES = 8

# table layout (elements) in the flat DRAM "tables" tensor
TBL = 255
BASE_A = 0                      # (s, kx)        -> 8*3 tables
BASE_S1 = 24 * TBL              # (s, m, kx)     -> 8*21
BASE_S2 = BASE_S1 + 168 * TBL
TBL_TOTAL = BASE_S2 + 168 * TBL  # 91800 elements

NSLOT = 45                      # band slots per sample: 3 A + 21 S1 + 21 S2
BANDW = 128                     # cols per band slot
BF = NSLOT * BANDW              # 5760

# row tilings (out_row_start, M, input_row_start)
AX_TILES = [(0, 126, -1), (126, 126, 125), (252, 126, 251), (378, 126, 377),
            (504, 8, 503)]
S7_TILES = [(0, 122, -3), (122, 122, 119), (244, 122, 241), (366, 122, 363),
            (488, 24, 485)]


def _sub_ap(base_ap, pattern, offset):
    """Custom access-pattern view: list of [step, count] pairs + elem offset."""
    a = base_ap.copy()
    v = a.ap
    v.clear()
    for p in pattern:
        v.append(list(p))
    a.offset = base_ap.offset + offset
    return a


def _slot_a(kx):
    return kx


def _slot_s1(m, kx):
    return 3 + m * KK + kx


def _slot_s2(m, kx):
    return 24 + m * KK + kx


def build_kernel(nc):
    x = nc.dram_tensor("x", [S, N, N], F32, kind="ExternalInput").ap()
    f = nc.dram_tensor("f", [S, N, N], F32, kind="ExternalInput").ap()
    ka = nc.dram_tensor("kernelA", [S, 9], F32, kind="ExternalInput").ap()
    fc_w1 = [nc.dram_tensor(f"fc{i}_w1", [100, 9], F32, kind="ExternalInput").ap()
             for i in (1, 2)]
    fc_b1 = [nc.dram_tensor(f"fc{i}_b1", [100], F32, kind="ExternalInput").ap()
             for i in (1, 2)]
    fc_w2 = [nc.dram_tensor(f"fc{i}_w2", [147, 100], F32, kind="ExternalInput").ap()
             for i in (1, 2)]
    fc_b2 = [nc.dram_tensor(f"fc{i}_b2", [147], F32, kind="ExternalInput").ap()
             for i in (1, 2)]
    out = nc.dram_tensor("out", [S, N, N], F32, kind="ExternalOutput").ap()

    with TileContext(nc) as tc:
        with (
            tc.tile_pool(name="dram", bufs=1, space="DRAM") as dpool,
            tc.tile_pool(name="const", bufs=1) as cpool,
            tc.tile_pool(name="mlp", bufs=1) as mpool,
            tc.tile_pool(name="bandf", bufs=1) as bfpool,
            tc.tile_pool(name="bands", bufs=2) as bpool,
            tc.tile_pool(name="xa", bufs=6) as xa_pool,
            tc.tile_pool(name="fr", bufs=4) as fr_pool,
            tc.tile_pool(name="rhs7", bufs=4) as rhs_pool,
            tc.tile_pool(name="stout", bufs=4) as st_pool,
            tc.tile_pool(name="psA", bufs=2, space="PSUM") as psA,
            tc.tile_pool(name="ps1", bufs=2, space="PSUM") as ps1,
            tc.tile_pool(name="ps2", bufs=2, space="PSUM") as ps2,
            tc.tile_pool(name="psx", bufs=2, space="PSUM") as psx,
        ):
            tables = dpool.tile([TBL_TOTAL], F32)
            r_dram = dpool.tile([S, N, N], F32)
            tmp_dram = dpool.tile([S, ML, N, N], F32)

            # ---- constants: anti-diagonal reversal matrix Rev[k,p]=d(k+p=127)
            rev = cpool.tile([128, 128], F32)
            nc.gpsimd.memset(rev, 0.0)
            nc.gpsimd.affine_select(
                out=rev, in_=rev, compare_op=mybir.AluOpType.not_equal,
                fill=1.0, base=-127, pattern=[[1, 128]], channel_multiplier=1)

            # ---- zero-fill tables
            zt = cpool.tile([120, 765], F32)
            nc.vector.memset(zt, 0.0)
            nc.sync.dma_start(_sub_ap(tables, [[765, 120], [1, 765]], 0), zt)

            # ---------------- MLP + weight staging ----------------
            ident = cpool.tile([128, 128], F32)
            nc.gpsimd.memset(ident, 0.0)
            nc.gpsimd.affine_select(
                out=ident, in_=ident, compare_op=mybir.AluOpType.not_equal,
                fill=1.0, base=0, pattern=[[-1, 128]], channel_multiplier=1)

            vT = mpool.tile([9, S], F32)
            nc.sync.dma_start(vT, ka.rearrange("s k -> k s"))

            w_sb = {}  # (layer i, map m) -> [49, S] conv weights
            for i in range(2):
                w1n = mpool.tile([100, 9], F32, name=f"w1n{i}")
                nc.sync.dma_start(w1n, fc_w1[i])
                W1T = mpool.tile([9, 100], F32, name=f"W1T{i}")
                t1 = psx.tile([9, 100], F32, name=f"t1_{i}", tag="aux")
                nc.tensor.transpose(t1, w1n, ident[:100, :100])
                nc.vector.tensor_copy(W1T, t1)

                b1 = mpool.tile([100, 1], F32, name=f"b1_{i}")
                nc.sync.dma_start(b1, fc_b1[i].unsqueeze(1))

                w2n_a = mpool.tile([128, 100], F32, name=f"w2na{i}")
                nc.sync.dma_start(w2n_a, fc_w2[i][0:128, :])
                w2n_b = mpool.tile([19, 100], F32, name=f"w2nb{i}")
                nc.sync.dma_start(w2n_b, fc_w2[i][128:147, :])
                W2T = mpool.tile([100, 147], F32, name=f"W2T{i}")
                tr_a = psx.tile([100, 128], F32, name=f"tra{i}", tag="aux")
                nc.tensor.transpose(tr_a, w2n_a, ident)
                nc.vector.tensor_copy(W2T[:, 0:128], tr_a)
                tr_b = psx.tile([100, 19], F32, name=f"trb{i}", tag="aux")
                nc.tensor.transpose(tr_b, w2n_b, ident[:19, :19])
                nc.vector.tensor_copy(W2T[:, 128:147], tr_b)

                h_pre = psx.tile([100, S], F32, name=f"hpre{i}", tag="aux")
                nc.tensor.matmul(h_pre, W1T, vT, start=True, stop=True)
                h = mpool.tile([100, S], F32, name=f"h{i}")
                nc.scalar.activation(
                    h, h_pre, mybir.ActivationFunctionType.Gelu, bias=b1)

                for m in range(ML):
                    b2m = mpool.tile([49, 1], F32, name=f"b2_{i}_{m}")
                    nc.sync.dma_start(
                        b2m, fc_b2[i][49 * m:49 * m + 49].unsqueeze(1))
                    wp = psx.tile([49, S], F32, name=f"wp{i}{m}", tag="aux")
                    nc.tensor.matmul(wp, W2T[:, 49 * m:49 * m + 49], h,
                                     start=True, stop=True)
                    wsb = mpool.tile([49, S], F32, name=f"w_{i}_{m}")
                    nc.scalar.activation(
                        wsb, wp, mybir.ActivationFunctionType.Identity,
                        bias=b2m)
                    w_sb[(i, m)] = wsb

            # scatter conv weights into zero-padded tables (SWDGE, flexible)
            # A tables: U[(s*3+kx)*255 + 127 - ky] = kernelA[s, ky, kx]
            for ky in range(3):
                nc.gpsimd.dma_start(
                    _sub_ap(tables, [[TBL, 3], [3 * TBL, S]],
                            BASE_A + 127 - ky),
                    vT[3 * ky:3 * ky + 3, :])
            # stage1/2: U[((s*3+m)*7+kx)*255 + 127 - ky] = w[i][s, m, ky, kx]
            for i, base in ((0, BASE_S1), (1, BASE_S2)):
                for m in range(ML):
                    for ky in range(KK):
                        nc.gpsimd.dma_start(
                            _sub_ap(tables, [[TBL, KK], [21 * TBL, S]],
                                    base + m * KK * TBL + 127 - ky),
                            w_sb[(i, m)][KK * ky:KK * ky + KK, :])

            # ---------------- main per-sample loop ----------------
            for s in range(S):
                # ---- flipped bands Bf[i, slot, m] = U_slot[i + m]
                bf = bfpool.tile([128, BF], F32, name=f"bf{s}", tag="bf")
                for (nslots, slot0, base) in (
                        (3, 0, BASE_A + s * 3 * TBL),
                        (21, 3, BASE_S1 + s * 21 * TBL),
                        (21, 24, BASE_S2 + s * 21 * TBL)):
                    nc.sync.dma_start(
                        _sub_ap(bf, [[BF, 128], [BANDW, nslots], [1, BANDW]],
                                slot0 * BANDW),
                        _sub_ap(tables, [[1, 128], [TBL, nslots], [1, BANDW]],
                                base))
                # ---- reverse partitions on PE: B[p] = Bf[127-p]
                bb = bpool.tile([128, BF], F32R if USE_F32R else F32, name=f"bb{s}", tag="bands")
                for c in range(0, BF, 512):
                    w = min(512, BF - c)
                    pr = psx.tile([128, 512], F32, name=f"pr{s}_{c}", tag="aux")
                    nc.tensor.matmul(pr[:, :w], rev, bf[:, c:c + w],
                                     start=True, stop=True)
                    nc.scalar.copy(bb[:, c:c + w], pr[:, :w])

                def band(slot, M):
                    return bb[:, slot * BANDW:slot * BANDW + M]

                # ---- Ax and r = f - Ax (126-row tiles) ----
                for (o0, M, row_start) in AX_TILES:
                    xt = xa_pool.tile([128, N + 2], F32R if USE_F32R else F32,
                                      name=f"xt{s}_{o0}", tag="xa")
                    if row_start + 128 > N:          # bottom tile: ones pad
                        nc.gpsimd.memset(xt.bitcast(F32), 1.0)
                        nd = N - row_start
                        nc.gpsimd.dma_start(xt[0:nd, 1:N + 1],
                                            x[s, row_start:N, :])
                        nc.gpsimd.memset(xt[0:nd, 0:1].bitcast(F32), 0.0)
                    else:
                        lo = max(0, row_start)
                        p0 = lo - row_start
                        if p0 > 0:
                            nc.gpsimd.memset(xt[0:p0, :].bitcast(F32), 0.0)
                        nc.gpsimd.dma_start(xt[p0:128, 1:N + 1],
                                            x[s, lo:row_start + 128, :])
                        nc.gpsimd.memset(xt[:, 0:1].bitcast(F32), 0.0)
                        nc.gpsimd.memset(xt[:, N + 1:N + 2].bitcast(F32), 1.0)
                    ps = psA.tile([M, N], F32, name=f"psA{s}_{o0}", tag="ax")
                    for kx in range(3):
                        nc.tensor.matmul(ps, _cast(band(_slot_a(kx), M)),
                                         _cast(xt[:, kx:kx + N]),
                                         start=(kx == 0), stop=(kx == 2))
                    ft = fr_pool.tile([126, N], F32, name=f"ft{s}_{o0}",
                                      tag="f")
                    nc.sync.dma_start(ft[:M, :], f[s, o0:o0 + M, :])
                    rt = fr_pool.tile([126, N], F32, name=f"rt{s}_{o0}",
                                      tag="r")
                    nc.vector.tensor_sub(rt[:M, :], ft[:M, :], ps)
                    nc.scalar.dma_start(r_dram[s, o0:o0 + M, :], rt[:M, :])

                # ---- stage 1: tmp_m = corr7(r, w1_m) ----
                for (o0, M, row_start) in S7_TILES:
                    rt7 = rhs_pool.tile([128, N + 6], F32R if USE_F32R else F32,
                                        name=f"rt7_{s}_{o0}", tag="rt7")
                    nc.gpsimd.memset(rt7.bitcast(F32), 0.0)
                    lo = max(0, row_start)
                    hi = min(N, row_start + 128)
                    nc.gpsimd.dma_start(
                        rt7[lo - row_start:hi - row_start, 3:N + 3],
                        r_dram[s, lo:hi, :])
                    tm3 = st_pool.tile([122, 3 * N], F32,
                                       name=f"tm3_{s}_{o0}", tag="tmp")
                    for m in range(ML):
                        ps = ps1.tile([M, N], F32, name=f"ps1_{s}_{o0}_{m}",
                                      tag="s1")
                        for kx in range(KK):
                            nc.tensor.matmul(ps, _cast(band(_slot_s1(m, kx), M)),
                                             _cast(rt7[:, kx:kx + N]),
                                             start=(kx == 0), stop=(kx == 6))
                        nc.vector.tensor_copy(tm3[:M, m * N:(m + 1) * N], ps)
                    # one store for all 3 maps: tmp_dram[s, :, o0:o0+M, :]
                    nc.scalar.dma_start(
                        _sub_ap(tmp_dram, [[N, M], [N * N, ML], [1, N]],
                                ((s * ML) * N + o0) * N),
                        _sub_ap(tm3, [[3 * N, M], [N, ML], [1, N]], 0))

                # ---- stage 2: G2 = sum_m corr7(tmp_m, w2_m); out = x + G2
                for (o0, M, row_start) in S7_TILES:
                    lo = max(0, row_start)
                    hi = min(N, row_start + 128)
                    tt = rhs_pool.tile([128, 3 * (N + 6)], F32R if USE_F32R else F32,
                                       name=f"tt{s}_{o0}", tag="tt")
                    nc.gpsimd.memset(tt.bitcast(F32), 0.0)
                    # one load for all 3 maps, each into its 518-block at col 3
                    nc.gpsimd.dma_start(
                        _sub_ap(tt, [[3 * (N + 6), hi - lo],
                                     [N + 6, ML], [1, N]],
                                (lo - row_start) * 3 * (N + 6) + 3),
                        _sub_ap(tmp_dram, [[N, hi - lo], [N * N, ML], [1, N]],
                                ((s * ML) * N + lo) * N))
                    pg = ps2.tile([M, N], F32, name=f"ps2_{s}_{o0}", tag="s2")
                    idx = 0
                    for m in range(ML):
                        for kx in range(KK):
                            nc.tensor.matmul(
                                pg, _cast(band(_slot_s2(m, kx), M)),
                                _cast(tt[:, m * (N + 6) + kx:m * (N + 6) + kx + N]),
                                start=(idx == 0), stop=(idx == 20))
                            idx += 1
                    x2 = fr_pool.tile([126, N], F32, name=f"x2_{s}_{o0}",
                                      tag="x2")
                    nc.sync.dma_start(x2[:M, :], x[s, o0:o0 + M, :])
                    ob = st_pool.tile([122, N], F32, name=f"ob{s}_{o0}",
                                      tag="ob")
                    nc.vector.tensor_add(ob[:M, :], x2[:M, :], pg)
                    nc.scalar.dma_start(out[s, o0:o0 + M, :], ob[:M, :])
    return nc


_CACHED = None


def _get_nc():
    global _CACHED
    if _CACHED is None:
        nc = bacc.Bacc("TRN2", debug=False, enable_asserts=False,
                       num_devices=NCORES)
        build_kernel(nc)
        nc.compile()
        _CACHED = nc
    return _CACHED


def make_in_maps(x, f, kernelA, fc1_w1, fc1_b1, fc1_w2, fc1_b2,
                 fc2_w1, fc2_b1, fc2_w2, fc2_b2):
    shared = {
        "fc1_w1": np.ascontiguousarray(fc1_w1, np.float32),
        "fc1_b1": np.ascontiguousarray(fc1_b1, np.float32),
        "fc1_w2": np.ascontiguousarray(fc1_w2, np.float32),
        "fc1_b2": np.ascontiguousarray(fc1_b2, np.float32),
        "fc2_w1": np.ascontiguousarray(fc2_w1, np.float32),
        "fc2_b1": np.ascontiguousarray(fc2_b1, np.float32),
        "fc2_w2": np.ascontiguousarray(fc2_w2, np.float32),
        "fc2_b2": np.ascontiguousarray(fc2_b2, np.float32),
    }
    in_maps = []
    for c in range(NCORES):
        sl = slice(S * c, S * (c + 1))
        in_maps.append({
            "x": np.ascontiguousarray(x[sl, 0], np.float32),
            "f": np.ascontiguousarray(f[sl, 0], np.float32),
            "kernelA": np.ascontiguousarray(
                kernelA[sl, 0].reshape(S, 9), np.float32),
            **shared,
        })
    return in_maps


def kernel(x, f, kernelA, fc1_w1, fc1_b1, fc1_w2, fc1_b2,
           fc2_w1, fc2_b1, fc2_w2, fc2_b2):
    x = np.asarray(x)
    nc = _get_nc()
    in_maps = make_in_maps(x, f, kernelA, fc1_w1, fc1_b1, fc1_w2, fc1_b2,
                           fc2_w1, fc2_b1, fc2_w2, fc2_b2)
    res = bass_utils.run_bass_kernel_spmd(
        nc, in_maps, core_ids=list(range(NCORES)))
    outs = [res.results[c]["out"] for c in range(NCORES)]
    full = np.concatenate(outs, axis=0).reshape(64, 1, N, N).astype(np.float32)
    return full



# revision 8
# speedup vs baseline: 1.0260x; 1.0260x over previous
"""MetaConvSmoother Trainium2 kernel (Bass/Tile), data-parallel over 8 NeuronCores.

v2: fully SBUF-resident pipeline (no DRAM round trips for r / tmp).

Per core (8 samples):
  - hypernet MLPs (9 -> 100 -> 147, exact gelu) on PE + ACT
  - per-sample conv kernels staged as zero-padded tables in DRAM
    (one 255-float table per (sample, slot)); bands loaded as
    overlapping windows Bf[p, m] = T[p + m] with all-positive strides.
  - parity trick: a window band consumed against an ASCENDING rhs
    produces a DESCENDING (partition-flipped) output and vice versa.
    Chain: Ax (REV band, asc->asc) -> r asc -> stage1 (window band,
    asc->desc) -> tmp desc -> stage2 (window band, desc->asc) -> out.
    Only the 3-slot A band needs a PE partition-reversal (1 small
    matmul per sample); stage1/stage2 bands are used as loaded.
  - r and tmp live in SBUF: producer PSUM chunks are written straight
    into the consumer's halo-tiled rhs tiles with partition-offset
    vector/scalar ops (no DMA round trip, PE stays warm).
  - DMA rings: SP (sync)   = band + x loads
               ACT (scalar)= f loads + out stores
               POOL (gpsimd)= x2 loads + table scatter/zero (SWDGE)
"""
import numpy as np

import concourse.bass as bass
import concourse.mybir as mybir
from concourse import bacc, bass_utils
from concourse.tile import TileContext

F32 = mybir.dt.float32
F32R = mybir.dt.float32r

S = 8          # samples per core
N = 512
ML = 3
KK = 7
NCORES = 8

TBL = 255                        # elements per slot table
NSLOT = 45                       # 3 A + 21 S1 + 21 S2
SLOT_A = 0
SLOT_S1 = 3
SLOT_S2 = 24
BANDW = 128
BF = NSLOT * BANDW               # 5760
TBL_TOTAL = S * NSLOT * TBL      # 91800

# Ax chunks: (o0, M); input x rows [o0-1, o0+127)
AX_CH = [(0, 126), (126, 126), (252, 126), (378, 126), (504, 8)]
# stage chunks: (o0, M); 122-row output chunks
S7_CH = [(0, 122), (122, 122), (244, 122), (366, 122), (488, 24)]
# rhs tile row starts (halo 3): tile k holds rows [RS[k], RS[k]+128)
RS = [-3, 119, 241, 363, 485]


def _sub_ap(base_ap, pattern, offset):
    """Custom access-pattern view: list of [step, count] pairs + elem offset."""
    a = base_ap.copy()
    v = a.ap
    v.clear()
    for p in pattern:
        v.append(list(p))
    a.offset = base_ap.offset + offset
    return a


def _overlaps_asc(o0, M):
    """(tile k, dest p0, src q0, nrows) for an ascending producer chunk
    [o0, o0+M) scattered into the asc rhs tiles (partition p = row-RS[k])."""
    out = []
    for k, rs in enumerate(RS):
        lo = max(o0, rs, 0)
        hi = min(o0 + M, rs + 128, N)
        if hi > lo:
            out.append((k, lo - rs, lo - o0, hi - lo))
    return out


def _overlaps_desc(o0, M, qoff):
    """(tile k, dest p0, src q0, nrows) for a descending producer chunk
    (psum part q = o0+121-row-qoff) into desc rhs tiles (p = RS[k]+127-row)."""
    out = []
    for k, rs in enumerate(RS):
        ck = rs + 127
        lo = max(o0, rs, 0)
        hi = min(o0 + M, rs + 128, N)
        if hi > lo:
            p0 = ck - (hi - 1)
            q0 = o0 + 121 - (hi - 1) - qoff
            out.append((k, p0, q0, hi - lo))
    return out


def build_kernel(nc):
    x = nc.dram_tensor("x", [S, N, N], F32, kind="ExternalInput").ap()
    f = nc.dram_tensor("f", [S, N, N], F32, kind="ExternalInput").ap()
    ka = nc.dram_tensor("kernelA", [S, 9], F32, kind="ExternalInput").ap()
    fc_w1 = [nc.dram_tensor(f"fc{i}_w1", [100, 9], F32, kind="ExternalInput").ap()
             for i in (1, 2)]
    fc_b1 = [nc.dram_tensor(f"fc{i}_b1", [100], F32, kind="ExternalInput").ap()
             for i in (1, 2)]
    fc_w2 = [nc.dram_tensor(f"fc{i}_w2", [147, 100], F32, kind="ExternalInput").ap()
             for i in (1, 2)]
    fc_b2 = [nc.dram_tensor(f"fc{i}_b2", [147], F32, kind="ExternalInput").ap()
             for i in (1, 2)]
    out = nc.dram_tensor("out", [S, N, N], F32, kind="ExternalOutput").ap()

    with TileContext(nc) as tc:
        with (
            tc.tile_pool(name="dram", bufs=1, space="DRAM") as dpool,
            tc.tile_pool(name="const", bufs=1) as cpool,
            tc.tile_pool(name="mlp", bufs=1) as mpool,
            tc.tile_pool(name="bands", bufs=2) as bpool,
            tc.tile_pool(name="banda", bufs=2) as bapool,
            tc.tile_pool(name="rt", bufs=2) as rtpool,
            tc.tile_pool(name="tp", bufs=2) as tppool,
            tc.tile_pool(name="xa", bufs=3) as xa_pool,
            tc.tile_pool(name="tm3", bufs=2) as tm3_pool,
            tc.tile_pool(name="fr", bufs=4) as fr_pool,
            tc.tile_pool(name="stout", bufs=4) as st_pool,
            tc.tile_pool(name="psA", bufs=2, space="PSUM") as psA,
            tc.tile_pool(name="ps1", bufs=3, space="PSUM") as ps1,
            tc.tile_pool(name="ps2", bufs=2, space="PSUM") as ps2,
            tc.tile_pool(name="psx", bufs=1, space="PSUM") as psx,
        ):
            tables = dpool.tile([TBL_TOTAL], F32)

            # ---- constants
            ident = cpool.tile([128, 128], F32)
            nc.gpsimd.memset(ident, 0.0)
            nc.gpsimd.affine_select(
                out=ident, in_=ident, compare_op=mybir.AluOpType.not_equal,
                fill=1.0, base=0, pattern=[[-1, 128]], channel_multiplier=1)
            # anti-diagonal reversal Rev[k,p] = d(k+p=127)
            rev_f = cpool.tile([128, 128], F32)
            nc.gpsimd.memset(rev_f, 0.0)
            nc.gpsimd.affine_select(
                out=rev_f, in_=rev_f, compare_op=mybir.AluOpType.not_equal,
                fill=1.0, base=-127, pattern=[[1, 128]], channel_multiplier=1)
            rev = cpool.tile([128, 128], F32R)
            nc.scalar.copy(rev, rev_f)   # round to f32r for the PE

            # ---- zero-fill tables (91800 = 120*765)
            zt = cpool.tile([120, 765], F32)
            nc.vector.memset(zt, 0.0)
            nc.gpsimd.dma_start(_sub_ap(tables, [[765, 120], [1, 765]], 0), zt)

            # ---------------- MLP + weight staging ----------------
            vT = mpool.tile([9, S], F32)
            nc.sync.dma_start(vT, ka.rearrange("s k -> k s"))

            w_sb = {}  # (layer i, map m) -> [49, S] conv weights
            for i in range(2):
                w1n = mpool.tile([100, 9], F32, name=f"w1n{i}")
                nc.sync.dma_start(w1n, fc_w1[i])
                W1T = mpool.tile([9, 100], F32, name=f"W1T{i}")
                t1 = psx.tile([9, 100], F32, name=f"t1_{i}", tag="aux")
                nc.tensor.transpose(t1, w1n, ident[:100, :100])
                nc.vector.tensor_copy(W1T, t1)

                b1 = mpool.tile([100, 1], F32, name=f"b1_{i}")
                nc.sync.dma_start(b1, fc_b1[i].unsqueeze(1))

                w2n_a = mpool.tile([128, 100], F32, name=f"w2na{i}")
                nc.sync.dma_start(w2n_a, fc_w2[i][0:128, :])
                w2n_b = mpool.tile([19, 100], F32, name=f"w2nb{i}")
                nc.sync.dma_start(w2n_b, fc_w2[i][128:147, :])
                W2T = mpool.tile([100, 147], F32, name=f"W2T{i}")
                tr_a = psx.tile([100, 128], F32, name=f"tra{i}", tag="aux")
                nc.tensor.transpose(tr_a, w2n_a, ident)
                nc.vector.tensor_copy(W2T[:, 0:128], tr_a)
                tr_b = psx.tile([100, 19], F32, name=f"trb{i}", tag="aux")
                nc.tensor.transpose(tr_b, w2n_b, ident[:19, :19])
                nc.vector.tensor_copy(W2T[:, 128:147], tr_b)

                h_pre = psx.tile([100, S], F32, name=f"hpre{i}", tag="aux")
                nc.tensor.matmul(h_pre, W1T, vT, start=True, stop=True)
                h = mpool.tile([100, S], F32, name=f"h{i}")
                nc.scalar.activation(
                    h, h_pre, mybir.ActivationFunctionType.Gelu, bias=b1)

                for m in range(ML):
                    b2m = mpool.tile([49, 1], F32, name=f"b2_{i}_{m}")
                    nc.sync.dma_start(
                        b2m, fc_b2[i][49 * m:49 * m + 49].unsqueeze(1))
                    wp = psx.tile([49, S], F32, name=f"wp{i}{m}", tag="aux")
                    nc.tensor.matmul(wp, W2T[:, 49 * m:49 * m + 49], h,
                                     start=True, stop=True)
                    wsb = mpool.tile([49, S], F32, name=f"w_{i}_{m}")
                    nc.scalar.activation(
                        wsb, wp, mybir.ActivationFunctionType.Identity,
                        bias=b2m)
                    w_sb[(i, m)] = wsb

            # ---- scatter conv weights into zero-padded tables (SWDGE)
            # table layout: [(s*45 + slot) * 255]; window read T[p+m].
            # A slots (0..2, per kx): T[127-ky] = A[ky,kx]  (REV band later)
            for ky in range(3):
                nc.gpsimd.dma_start(
                    _sub_ap(tables, [[TBL, 3], [NSLOT * TBL, S]],
                            SLOT_A * TBL + 127 - ky),
                    vT[3 * ky:3 * ky + 3, :])
            # stage1 slots (3 + m*7 + kx): T[121+ky] = w1[ky,kx,m]
            # stage2 slots (24 + m*7 + kx): T[127-ky] = w2[ky,kx,m]
            for (i, base, off0, dky) in ((0, SLOT_S1, 121, 1),
                                         (1, SLOT_S2, 127, -1)):
                for m in range(ML):
                    for ky in range(KK):
                        nc.gpsimd.dma_start(
                            _sub_ap(tables, [[TBL, KK], [NSLOT * TBL, S]],
                                    (base + m * KK) * TBL + off0 + dky * ky),
                            w_sb[(i, m)][KK * ky:KK * ky + KK, :])

            # ---------------- main per-sample loop ----------------
            for s in range(S):
                # ---- window bands: bb[p, slot*128+m] = T_slot[p+m]
                bb = bpool.tile([128, BF], F32R, name=f"bb{s}", tag="bb")
                nc.sync.dma_start(
                    _sub_ap(bb, [[BF, 128], [BANDW, NSLOT], [1, BANDW]], 0),
                    _sub_ap(tables.bitcast(F32R),
                            [[1, 128], [TBL, NSLOT], [1, BANDW]],
                            s * NSLOT * TBL))

                def band(slot, m0, m1):
                    return bb[:, slot * BANDW + m0:slot * BANDW + m1]

                # ---- A band: reverse partitions on PE (3 slots = 384 cols)
                ba = bapool.tile([128, 3 * BANDW], F32R, name=f"ba{s}",
                                 tag="ba")
                pr = psx.tile([128, 3 * BANDW], F32, name=f"pr{s}", tag="aux")
                nc.tensor.matmul(pr, rev, bb[:, 0:3 * BANDW],
                                 start=True, stop=True)
                nc.scalar.copy(ba, pr)

                # ---- stage-1 rhs tiles (r, ASC: p = row - RS[k]) and
                # ---- stage-2 rhs tiles (tmp, DESC: p = RS[k]+127 - row).
                # Data DMAs only ever write the interior; the zero halo
                # (cols 0:3 / 515:518, out-of-image partitions) is written
                # once per physical buffer (samples 0 and 1 cover bufs=2).
                rt, tp = [], []
                for k in range(5):
                    t = rtpool.tile([128, N + 6], F32R, name=f"rt{s}_{k}",
                                    tag=f"rt{k}")
                    rt.append(t)
                    if s < 2:
                        nc.gpsimd.memset(t.bitcast(F32), 0.0)
                    t2 = tppool.tile([128, ML * (N + 6)], F32R,
                                     name=f"tp{s}_{k}", tag=f"tp{k}")
                    tp.append(t2)
                    if s < 2:
                        nc.gpsimd.memset(t2.bitcast(F32), 0.0)

                # ---- Ax chunks; r = f - Ax written into rt tiles
                for j, (o0, M) in enumerate(AX_CH):
                    row_start = o0 - 1
                    xt = xa_pool.tile([128, N + 2], F32R,
                                      name=f"xt{s}_{j}", tag="xa")
                    if row_start + 128 > N:          # bottom: ones pad
                        nc.gpsimd.memset(xt.bitcast(F32), 1.0)
                        nd = N - row_start
                        nc.sync.dma_start(
                            xt[0:nd, 1:N + 1],
                            x.bitcast(F32R)[s, row_start:N, :])
                        nc.gpsimd.memset(xt[0:nd, 0:1].bitcast(F32), 0.0)
                    else:
                        lo = max(0, row_start)
                        p0 = lo - row_start
                        if p0 > 0:
                            nc.gpsimd.memset(xt[0:p0, :].bitcast(F32), 0.0)
                        nc.sync.dma_start(
                            xt[p0:128, 1:N + 1],
                            x.bitcast(F32R)[s, lo:row_start + 128, :])
                        nc.gpsimd.memset(xt[:, 0:1].bitcast(F32), 0.0)
                        nc.gpsimd.memset(xt[:, N + 1:N + 2].bitcast(F32), 1.0)
                    ps = psA.tile([M, N], F32, name=f"psA{s}_{j}", tag="ax")
                    for kx in range(3):
                        nc.tensor.matmul(ps, ba[:, kx * BANDW:kx * BANDW + M],
                                         xt[:, kx:kx + N],
                                         start=(kx == 0), stop=(kx == 2))
                    ft = fr_pool.tile([126, N], F32, name=f"ft{s}_{j}",
                                      tag="f")
                    nc.scalar.dma_start(ft[:M, :], f[s, o0:o0 + M, :])
                    rf = fr_pool.tile([126, N], F32, name=f"rf{s}_{j}",
                                      tag="rf")
                    nc.vector.tensor_sub(rf[:M, :], ft[:M, :], ps[:M, :])
                    for (k, p0, q0, n) in _overlaps_asc(o0, M):
                        nc.scalar.dma_start(
                            _sub_ap(rt[k], [[N + 6, n], [1, N]],
                                    p0 * (N + 6) + 3),
                            _sub_ap(rf.bitcast(F32R), [[N, n], [1, N]],
                                    q0 * N))

                # ---- stage 1: tmp (desc) from r (asc)
                for j, (o0, M) in enumerate(S7_CH):
                    qoff = 98 if M < 122 else 0   # lhsT col slice for c4
                    tm3 = tm3_pool.tile([122, ML * N], F32,
                                        name=f"tm3_{s}_{j}", tag="tm3")
                    for m in range(ML):
                        ps_ = ps1.tile([122, N], F32, name=f"ps1_{s}_{j}_{m}",
                                       tag="s1")
                        for kx in range(KK):
                            nc.tensor.matmul(
                                ps_[:M, :],
                                band(SLOT_S1 + m * KK + kx, qoff, qoff + M),
                                rt[j][:, kx:kx + N],
                                start=(kx == 0), stop=(kx == KK - 1))
                        nc.vector.tensor_copy(
                            tm3[:M, m * N:(m + 1) * N], ps_[:M, :])
                    for (k, p0, q0, n) in _overlaps_desc(o0, M, qoff):
                        eng = nc.sync if n > 8 else nc.scalar
                        eng.dma_start(
                            _sub_ap(tp[k],
                                    [[ML * (N + 6), n], [N + 6, ML], [1, N]],
                                    p0 * ML * (N + 6) + 3),
                            _sub_ap(tm3.bitcast(F32R),
                                    [[ML * N, n], [N, ML], [1, N]],
                                    q0 * ML * N))

                # ---- stage 2: G2 (asc) from tmp (desc); out = x + G2
                for j, (o0, M) in enumerate(S7_CH):
                    pg = ps2.tile([122, N], F32, name=f"ps2_{s}_{j}", tag="s2")
                    idx = 0
                    for m in range(ML):
                        for kx in range(KK):
                            nc.tensor.matmul(
                                pg[:M, :],
                                band(SLOT_S2 + m * KK + kx, 0, M),
                                tp[j][:, m * (N + 6) + kx:m * (N + 6) + kx + N],
                                start=(idx == 0), stop=(idx == 20))
                            idx += 1
                    x2 = fr_pool.tile([122, N], F32, name=f"x2_{s}_{j}",
                                      tag="x2")
                    nc.gpsimd.dma_start(x2[:M, :], x[s, o0:o0 + M, :])
                    ob = st_pool.tile([122, N], F32, name=f"ob{s}_{j}",
                                      tag="ob")
                    nc.vector.tensor_add(ob[:M, :], x2[:M, :], pg[:M, :])
                    nc.scalar.dma_start(out[s, o0:o0 + M, :], ob[:M, :])
    return nc


_CACHED = None


def _get_nc():
    global _CACHED
    if _CACHED is None:
        nc = bacc.Bacc("TRN2", debug=False, enable_asserts=False,
                       num_devices=NCORES)
        build_kernel(nc)
        nc.compile()
        _CACHED = nc
    return _CACHED


def make_in_maps(x, f, kernelA, fc1_w1, fc1_b1, fc1_w2, fc1_b2,
                 fc2_w1, fc2_b1, fc2_w2, fc2_b2):
    shared = {
        "fc1_w1": np.ascontiguousarray(fc1_w1, np.float32),
        "fc1_b1": np.ascontiguousarray(fc1_b1, np.float32),
        "fc1_w2": np.ascontiguousarray(fc1_w2, np.float32),
        "fc1_b2": np.ascontiguousarray(fc1_b2, np.float32),
        "fc2_w1": np.ascontiguousarray(fc2_w1, np.float32),
        "fc2_b1": np.ascontiguousarray(fc2_b1, np.float32),
        "fc2_w2": np.ascontiguousarray(fc2_w2, np.float32),
        "fc2_b2": np.ascontiguousarray(fc2_b2, np.float32),
    }
    in_maps = []
    for c in range(NCORES):
        sl = slice(S * c, S * (c + 1))
        in_maps.append({
            "x": np.ascontiguousarray(x[sl, 0], np.float32),
            "f": np.ascontiguousarray(f[sl, 0], np.float32),
            "kernelA": np.ascontiguousarray(
                kernelA[sl, 0].reshape(S, 9), np.float32),
            **shared,
        })
    return in_maps


def kernel(x, f, kernelA, fc1_w1, fc1_b1, fc1_w2, fc1_b2,
           fc2_w1, fc2_b1, fc2_w2, fc2_b2):
    x = np.asarray(x)
    nc = _get_nc()
    in_maps = make_in_maps(x, f, kernelA, fc1_w1, fc1_b1, fc1_w2, fc1_b2,
                           fc2_w1, fc2_b1, fc2_w2, fc2_b2)
    res = bass_utils.run_bass_kernel_spmd(
        nc, in_maps, core_ids=list(range(NCORES)))
    outs = [res.results[c]["out"] for c in range(NCORES)]
    full = np.concatenate(outs, axis=0).reshape(64, 1, N, N).astype(np.float32)
    return full


# revision 10
# speedup vs baseline: 1.0481x; 1.0215x over previous
"""MetaConvSmoother Trainium2 kernel (Bass/Tile), data-parallel over 8 NeuronCores.

v2: fully SBUF-resident pipeline (no DRAM round trips for r / tmp).

Per core (8 samples):
  - hypernet MLPs (9 -> 100 -> 147, exact gelu) on PE + ACT
  - per-sample conv kernels staged as zero-padded tables in DRAM
    (one 255-float table per (sample, slot)); bands loaded as
    overlapping windows Bf[p, m] = T[p + m] with all-positive strides.
  - parity trick: a window band consumed against an ASCENDING rhs
    produces a DESCENDING (partition-flipped) output and vice versa.
    Chain: Ax (REV band, asc->asc) -> r asc -> stage1 (window band,
    asc->desc) -> tmp desc -> stage2 (window band, desc->asc) -> out.
    Only the 3-slot A band needs a PE partition-reversal (1 small
    matmul per sample); stage1/stage2 bands are used as loaded.
  - r and tmp live in SBUF: producer PSUM chunks are written straight
    into the consumer's halo-tiled rhs tiles with partition-offset
    vector/scalar ops (no DMA round trip, PE stays warm).
  - DMA rings: SP (sync)   = band + x loads
               ACT (scalar)= f loads + out stores
               POOL (gpsimd)= x2 loads + table scatter/zero (SWDGE)
"""
import numpy as np

import concourse.bass as bass
import concourse.mybir as mybir
from concourse import bacc, bass_utils
from concourse.tile import TileContext

F32 = mybir.dt.float32
F32R = mybir.dt.float32r

S = 8          # samples per core
N = 512
ML = 3
KK = 7
NCORES = 8

TBL = 255                        # elements per slot table
NSLOT = 45                       # 3 A + 21 S1 + 21 S2
SLOT_A = 0
SLOT_S1 = 3
SLOT_S2 = 24
BANDW = 128
BF = NSLOT * BANDW               # 5760
TBL_TOTAL = S * NSLOT * TBL      # 91800

# Ax chunks: (o0, M); input x rows [o0-1, o0+127)
AX_CH = [(0, 126), (126, 126), (252, 126), (378, 126), (504, 8)]
# stage chunks: (o0, M); 122-row output chunks
S7_CH = [(0, 122), (122, 122), (244, 122), (366, 122), (488, 24)]
# rhs tile row starts (halo 3): tile k holds rows [RS[k], RS[k]+128)
RS = [-3, 119, 241, 363, 485]


def _sub_ap(base_ap, pattern, offset):
    """Custom access-pattern view: list of [step, count] pairs + elem offset."""
    a = base_ap.copy()
    v = a.ap
    v.clear()
    for p in pattern:
        v.append(list(p))
    a.offset = base_ap.offset + offset
    return a


def _overlaps_asc(o0, M):
    """(tile k, dest p0, src q0, nrows) for an ascending producer chunk
    [o0, o0+M) scattered into the asc rhs tiles (partition p = row-RS[k])."""
    out = []
    for k, rs in enumerate(RS):
        lo = max(o0, rs, 0)
        hi = min(o0 + M, rs + 128, N)
        if hi > lo:
            out.append((k, lo - rs, lo - o0, hi - lo))
    return out


def _overlaps_desc(o0, M, qoff):
    """(tile k, dest p0, src q0, nrows) for a descending producer chunk
    (psum part q = o0+121-row-qoff) into desc rhs tiles (p = RS[k]+127-row)."""
    out = []
    for k, rs in enumerate(RS):
        ck = rs + 127
        lo = max(o0, rs, 0)
        hi = min(o0 + M, rs + 128, N)
        if hi > lo:
            p0 = ck - (hi - 1)
            q0 = o0 + 121 - (hi - 1) - qoff
            out.append((k, p0, q0, hi - lo))
    return out


def build_kernel(nc):
    x = nc.dram_tensor("x", [S, N, N], F32, kind="ExternalInput").ap()
    f = nc.dram_tensor("f", [S, N, N], F32, kind="ExternalInput").ap()
    ka = nc.dram_tensor("kernelA", [S, 9], F32, kind="ExternalInput").ap()
    fc_w1 = [nc.dram_tensor(f"fc{i}_w1", [100, 9], F32, kind="ExternalInput").ap()
             for i in (1, 2)]
    fc_b1 = [nc.dram_tensor(f"fc{i}_b1", [100], F32, kind="ExternalInput").ap()
             for i in (1, 2)]
    fc_w2 = [nc.dram_tensor(f"fc{i}_w2", [147, 100], F32, kind="ExternalInput").ap()
             for i in (1, 2)]
    fc_b2 = [nc.dram_tensor(f"fc{i}_b2", [147], F32, kind="ExternalInput").ap()
             for i in (1, 2)]
    out = nc.dram_tensor("out", [S, N, N], F32, kind="ExternalOutput").ap()

    with TileContext(nc) as tc:
        with (
            tc.tile_pool(name="dram", bufs=1, space="DRAM") as dpool,
            tc.tile_pool(name="const", bufs=1) as cpool,
            tc.tile_pool(name="mlp", bufs=1) as mpool,
            tc.tile_pool(name="bands", bufs=2) as bpool,
            tc.tile_pool(name="banda", bufs=2) as bapool,
            tc.tile_pool(name="rt", bufs=2) as rtpool,
            tc.tile_pool(name="tp", bufs=2) as tppool,
            tc.tile_pool(name="xa", bufs=2) as xa_pool,
            tc.tile_pool(name="tm3", bufs=2) as tm3_pool,
            tc.tile_pool(name="fr", bufs=3) as fr_pool,
            tc.tile_pool(name="stout", bufs=2) as st_pool,
            tc.tile_pool(name="psA", bufs=2, space="PSUM") as psA,
            tc.tile_pool(name="ps1", bufs=3, space="PSUM") as ps1,
            tc.tile_pool(name="ps2", bufs=2, space="PSUM") as ps2,
            tc.tile_pool(name="psx", bufs=1, space="PSUM") as psx,
        ):
            tables = dpool.tile([TBL_TOTAL], F32)

            # ---- constants
            ident = cpool.tile([128, 128], F32)
            nc.gpsimd.memset(ident, 0.0)
            nc.gpsimd.affine_select(
                out=ident, in_=ident, compare_op=mybir.AluOpType.not_equal,
                fill=1.0, base=0, pattern=[[-1, 128]], channel_multiplier=1)
            # anti-diagonal reversal Rev[k,p] = d(k+p=127)
            rev_f = cpool.tile([128, 128], F32)
            nc.gpsimd.memset(rev_f, 0.0)
            nc.gpsimd.affine_select(
                out=rev_f, in_=rev_f, compare_op=mybir.AluOpType.not_equal,
                fill=1.0, base=-127, pattern=[[1, 128]], channel_multiplier=1)
            rev = cpool.tile([128, 128], F32R)
            nc.scalar.copy(rev, rev_f)   # round to f32r for the PE

            # ---- zero-fill tables (91800 = 120*765)
            zt = cpool.tile([120, 765], F32)
            nc.vector.memset(zt, 0.0)
            nc.gpsimd.dma_start(_sub_ap(tables, [[765, 120], [1, 765]], 0), zt)

            # ---------------- MLP + weight staging ----------------
            vT = mpool.tile([9, S], F32)
            nc.sync.dma_start(vT, ka.rearrange("s k -> k s"))

            w_sb = {}  # (layer i, map m) -> [49, S] conv weights
            for i in range(2):
                w1n = mpool.tile([100, 9], F32, name=f"w1n{i}")
                nc.sync.dma_start(w1n, fc_w1[i])
                W1T = mpool.tile([9, 100], F32, name=f"W1T{i}")
                t1 = psx.tile([9, 100], F32, name=f"t1_{i}", tag="aux")
                nc.tensor.transpose(t1, w1n, ident[:100, :100])
                nc.vector.tensor_copy(W1T, t1)

                b1 = mpool.tile([100, 1], F32, name=f"b1_{i}")
                nc.sync.dma_start(b1, fc_b1[i].unsqueeze(1))

                w2n_a = mpool.tile([128, 100], F32, name=f"w2na{i}")
                nc.sync.dma_start(w2n_a, fc_w2[i][0:128, :])
                w2n_b = mpool.tile([19, 100], F32, name=f"w2nb{i}")
                nc.sync.dma_start(w2n_b, fc_w2[i][128:147, :])
                W2T = mpool.tile([100, 147], F32, name=f"W2T{i}")
                tr_a = psx.tile([100, 128], F32, name=f"tra{i}", tag="aux")
                nc.tensor.transpose(tr_a, w2n_a, ident)
                nc.vector.tensor_copy(W2T[:, 0:128], tr_a)
                tr_b = psx.tile([100, 19], F32, name=f"trb{i}", tag="aux")
                nc.tensor.transpose(tr_b, w2n_b, ident[:19, :19])
                nc.vector.tensor_copy(W2T[:, 128:147], tr_b)

                h_pre = psx.tile([100, S], F32, name=f"hpre{i}", tag="aux")
                nc.tensor.matmul(h_pre, W1T, vT, start=True, stop=True)
                h = mpool.tile([100, S], F32, name=f"h{i}")
                nc.scalar.activation(
                    h, h_pre, mybir.ActivationFunctionType.Gelu, bias=b1)

                for m in range(ML):
                    b2m = mpool.tile([49, 1], F32, name=f"b2_{i}_{m}")
                    nc.sync.dma_start(
                        b2m, fc_b2[i][49 * m:49 * m + 49].unsqueeze(1))
                    wp = psx.tile([49, S], F32, name=f"wp{i}{m}", tag="aux")
                    nc.tensor.matmul(wp, W2T[:, 49 * m:49 * m + 49], h,
                                     start=True, stop=True)
                    wsb = mpool.tile([49, S], F32, name=f"w_{i}_{m}")
                    nc.scalar.activation(
                        wsb, wp, mybir.ActivationFunctionType.Identity,
                        bias=b2m)
                    w_sb[(i, m)] = wsb

            # ---- scatter conv weights into zero-padded tables (SWDGE)
            # table layout: [(s*45 + slot) * 255]; window read T[p+m].
            # A slots (0..2, per kx): T[127-ky] = A[ky,kx]  (REV band later)
            for ky in range(3):
                nc.gpsimd.dma_start(
                    _sub_ap(tables, [[TBL, 3], [NSLOT * TBL, S]],
                            SLOT_A * TBL + 127 - ky),
                    vT[3 * ky:3 * ky + 3, :])
            # stage1 slots (3 + m*7 + kx): T[121+ky] = w1[ky,kx,m]
            # stage2 slots (24 + m*7 + kx): T[127-ky] = w2[ky,kx,m]
            for (i, base, off0, dky) in ((0, SLOT_S1, 121, 1),
                                         (1, SLOT_S2, 127, -1)):
                for m in range(ML):
                    for ky in range(KK):
                        nc.gpsimd.dma_start(
                            _sub_ap(tables, [[TBL, KK], [NSLOT * TBL, S]],
                                    (base + m * KK) * TBL + off0 + dky * ky),
                            w_sb[(i, m)][KK * ky:KK * ky + KK, :])

            # ---------------- main per-sample loop (software-pipelined) ----
            # PE order per iteration s: rev(s), Ax(s), stage2(s-1), stage1(s)
            # so PE has ready matmul work while r/tmp scatter DMAs land.
            # All DRAM loads for s+1 are issued mid-iteration s.

            bb_t, ba_t, rt_t, tp_t, xt_t = {}, {}, {}, {}, {}

            def band(s, slot, m0, m1):
                b = bb_t[s]
                return b[:, slot * BANDW + m0:slot * BANDW + m1]

            def emit_band_load(s):
                bb = bpool.tile([128, BF], F32R, name=f"bb{s}", tag="bb")
                bb_t[s] = bb
                half = 23 * BANDW
                nc.sync.dma_start(
                    _sub_ap(bb, [[BF, 128], [BANDW, 23], [1, BANDW]], 0),
                    _sub_ap(tables.bitcast(F32R),
                            [[1, 128], [TBL, 23], [1, BANDW]],
                            s * NSLOT * TBL))
                nc.scalar.dma_start(
                    _sub_ap(bb, [[BF, 128], [BANDW, 22], [1, BANDW]], half),
                    _sub_ap(tables.bitcast(F32R),
                            [[1, 128], [TBL, 22], [1, BANDW]],
                            s * NSLOT * TBL + 23 * TBL))

            def emit_x_load(s):
                xts = []
                for j, (o0, M) in enumerate(AX_CH):
                    row_start = o0 - 1
                    xt = xa_pool.tile([128, N + 2], F32R,
                                      name=f"xt{s}_{j}", tag=f"xa{j}")
                    xts.append(xt)
                    if row_start + 128 > N:          # bottom: ones pad
                        nc.gpsimd.memset(xt.bitcast(F32), 1.0)
                        nd = N - row_start
                        nc.sync.dma_start(
                            xt[0:nd, 1:N + 1],
                            x.bitcast(F32R)[s, row_start:N, :])
                        nc.gpsimd.memset(xt[0:nd, 0:1].bitcast(F32), 0.0)
                    else:
                        lo = max(0, row_start)
                        p0 = lo - row_start
                        if p0 > 0:
                            nc.gpsimd.memset(xt[0:p0, :].bitcast(F32), 0.0)
                        nc.sync.dma_start(
                            xt[p0:128, 1:N + 1],
                            x.bitcast(F32R)[s, lo:row_start + 128, :])
                        nc.gpsimd.memset(xt[:, 0:1].bitcast(F32), 0.0)
                        nc.gpsimd.memset(xt[:, N + 1:N + 2].bitcast(F32), 1.0)
                xt_t[s] = xts

            def emit_rev(s):
                ba = bapool.tile([128, 3 * BANDW], F32R, name=f"ba{s}",
                                 tag="ba")
                ba_t[s] = ba
                pr = psx.tile([128, 3 * BANDW], F32, name=f"pr{s}", tag="aux")
                nc.tensor.matmul(pr, rev, bb_t[s][:, 0:3 * BANDW],
                                 start=True, stop=True)
                nc.scalar.copy(ba, pr)

            def emit_rhs_tiles(s):
                rt, tp = [], []
                for k in range(5):
                    t = rtpool.tile([128, N + 6], F32R, name=f"rt{s}_{k}",
                                    tag=f"rt{k}")
                    rt.append(t)
                    if s < 2:
                        nc.gpsimd.memset(t.bitcast(F32), 0.0)
                    t2 = tppool.tile([128, ML * (N + 6)], F32R,
                                     name=f"tp{s}_{k}", tag=f"tp{k}")
                    tp.append(t2)
                    if s < 2:
                        nc.gpsimd.memset(t2.bitcast(F32), 0.0)
                rt_t[s], tp_t[s] = rt, tp

            def emit_ax(s):
                ba, rt = ba_t[s], rt_t[s]
                for j, (o0, M) in enumerate(AX_CH):
                    xt = xt_t[s][j]
                    ft = fr_pool.tile([126, N], F32, name=f"ft{s}_{j}",
                                      tag="f")
                    nc.scalar.dma_start(ft[:M, :], f[s, o0:o0 + M, :])
                    ps = psA.tile([M, N], F32, name=f"psA{s}_{j}", tag="ax")
                    for kx in range(3):
                        nc.tensor.matmul(ps, ba[:, kx * BANDW:kx * BANDW + M],
                                         xt[:, kx:kx + N],
                                         start=(kx == 0), stop=(kx == 2))
                    rf = fr_pool.tile([126, N], F32, name=f"rf{s}_{j}",
                                      tag="rf")
                    nc.vector.tensor_sub(rf[:M, :], ft[:M, :], ps[:M, :])
                    for (k, p0, q0, n) in _overlaps_asc(o0, M):
                        eng = nc.sync if n > 32 else nc.gpsimd
                        eng.dma_start(
                            _sub_ap(rt[k], [[N + 6, n], [1, N]],
                                    p0 * (N + 6) + 3),
                            _sub_ap(rf.bitcast(F32R), [[N, n], [1, N]],
                                    q0 * N))

            def emit_stage1(s):
                rt, tp = rt_t[s], tp_t[s]
                for j, (o0, M) in enumerate(S7_CH):
                    qoff = 98 if M < 122 else 0   # lhsT col slice for c4
                    tm3 = tm3_pool.tile([122, ML * N], F32,
                                        name=f"tm3_{s}_{j}", tag="tm3")
                    for m in range(ML):
                        ps_ = ps1.tile([122, N], F32, name=f"ps1_{s}_{j}_{m}",
                                       tag="s1")
                        for kx in range(KK):
                            nc.tensor.matmul(
                                ps_[:M, :],
                                band(s, SLOT_S1 + m * KK + kx, qoff, qoff + M),
                                rt[j][:, kx:kx + N],
                                start=(kx == 0), stop=(kx == KK - 1))
                        nc.vector.tensor_copy(
                            tm3[:M, m * N:(m + 1) * N], ps_[:M, :])
                    for (k, p0, q0, n) in _overlaps_desc(o0, M, qoff):
                        eng = (nc.sync if j % 2 == 0 else
                               nc.scalar) if n > 32 else nc.gpsimd
                        eng.dma_start(
                            _sub_ap(tp[k],
                                    [[ML * (N + 6), n], [N + 6, ML], [1, N]],
                                    p0 * ML * (N + 6) + 3),
                            _sub_ap(tm3.bitcast(F32R),
                                    [[ML * N, n], [N, ML], [1, N]],
                                    q0 * ML * N))

            def emit_stage2(s):
                tp = tp_t[s]
                for j, (o0, M) in enumerate(S7_CH):
                    pg = ps2.tile([122, N], F32, name=f"ps2_{s}_{j}", tag="s2")
                    idx = 0
                    for m in range(ML):
                        for kx in range(KK):
                            nc.tensor.matmul(
                                pg[:M, :],
                                band(s, SLOT_S2 + m * KK + kx, 0, M),
                                tp[j][:, m * (N + 6) + kx:
                                      m * (N + 6) + kx + N],
                                start=(idx == 0), stop=(idx == 20))
                            idx += 1
                    x2 = fr_pool.tile([122, N], F32, name=f"x2_{s}_{j}",
                                      tag="x2")
                    nc.gpsimd.dma_start(x2[:M, :], x[s, o0:o0 + M, :])
                    ob = st_pool.tile([122, N], F32, name=f"ob{s}_{j}",
                                      tag="ob")
                    nc.vector.tensor_add(ob[:M, :], x2[:M, :], pg[:M, :])
                    nc.scalar.dma_start(out[s, o0:o0 + M, :], ob[:M, :])
                del bb_t[s], ba_t[s], rt_t[s], tp_t[s], xt_t[s]

            # prologue
            emit_band_load(0)
            emit_x_load(0)
            for s in range(S):
                emit_rev(s)
                emit_rhs_tiles(s)
                emit_ax(s)
                # prefetch next iteration inputs
                if s + 1 < S:
                    emit_band_load(s + 1)
                    emit_x_load(s + 1)
                if s >= 1:
                    emit_stage2(s - 1)
                emit_stage1(s)
            emit_stage2(S - 1)
    return nc


_CACHED = None


def _get_nc():
    global _CACHED
    if _CACHED is None:
        nc = bacc.Bacc("TRN2", debug=False, enable_asserts=False,
                       num_devices=NCORES)
        build_kernel(nc)
        nc.compile()
        _CACHED = nc
    return _CACHED


def make_in_maps(x, f, kernelA, fc1_w1, fc1_b1, fc1_w2, fc1_b2,
                 fc2_w1, fc2_b1, fc2_w2, fc2_b2):
    shared = {
        "fc1_w1": np.ascontiguousarray(fc1_w1, np.float32),
        "fc1_b1": np.ascontiguousarray(fc1_b1, np.float32),
        "fc1_w2": np.ascontiguousarray(fc1_w2, np.float32),
        "fc1_b2": np.ascontiguousarray(fc1_b2, np.float32),
        "fc2_w1": np.ascontiguousarray(fc2_w1, np.float32),
        "fc2_b1": np.ascontiguousarray(fc2_b1, np.float32),
        "fc2_w2": np.ascontiguousarray(fc2_w2, np.float32),
        "fc2_b2": np.ascontiguousarray(fc2_b2, np.float32),
    }
    in_maps = []
    for c in range(NCORES):
        sl = slice(S * c, S * (c + 1))
        in_maps.append({
            "x": np.ascontiguousarray(x[sl, 0], np.float32),
            "f": np.ascontiguousarray(f[sl, 0], np.float32),
            "kernelA": np.ascontiguousarray(
                kernelA[sl, 0].reshape(S, 9), np.float32),
            **shared,
        })
    return in_maps


def kernel(x, f, kernelA, fc1_w1, fc1_b1, fc1_w2, fc1_b2,
           fc2_w1, fc2_b1, fc2_w2, fc2_b2):
    x = np.asarray(x)
    nc = _get_nc()
    in_maps = make_in_maps(x, f, kernelA, fc1_w1, fc1_b1, fc1_w2, fc1_b2,
                           fc2_w1, fc2_b1, fc2_w2, fc2_b2)
    res = bass_utils.run_bass_kernel_spmd(
        nc, in_maps, core_ids=list(range(NCORES)))
    outs = [res.results[c]["out"] for c in range(NCORES)]
    full = np.concatenate(outs, axis=0).reshape(64, 1, N, N).astype(np.float32)
    return full


# revision 15
# speedup vs baseline: 1.0698x; 1.0207x over previous
"""MetaConvSmoother Trainium2 kernel (Bass/Tile), data-parallel over 8 NeuronCores.

v4: SBUF-resident pipeline, software-pipelined across samples.

Per core (8 samples):
  - hypernet MLPs (9 -> 100 -> 147, exact gelu) on PE + ACT
  - per-sample conv kernels staged as zero-padded tables in DRAM;
    bands loaded as overlapping windows Bf[p, m] = T[p + m] with
    all-positive strides.  fc2_w2/fc2_b2 arrive ROW-FLIPPED in ky
    (host-side) so every table scatter is an ascending batched DMA;
    kernelA_flip likewise feeds the A tables.
  - parity trick: window band on ASCENDING rhs -> DESCENDING output and
    vice versa.  Chain: Ax (PE-reversed A band, asc->asc) -> r asc ->
    stage1 (window, asc->desc) -> tmp desc -> stage2 (window,
    desc->asc) -> out.  Only the 3-slot A band is PE-reversed.
  - r and tmp live in SBUF: PSUM chunks go through base-0 staging
    tiles (DVE) then SBUF->SBUF DMA scatters into halo-tiled rhs
    tiles (compute engines cannot address partitions off 32-alignment,
    DMA can).
  - out = x + G2 accumulated ON THE PE: const double-identity bands
    (D_full / D_clip) add the x tiles into the stage-2 PSUM, removing
    the x2 reload and its DVE dependency chain.
  - per-iteration PE order: rev(s), Ax(s), stage2(s-1), stage1(s) so
    scatter latencies hide behind ready matmul work.
"""
import numpy as np

import concourse.bass as bass
import concourse.mybir as mybir
from concourse import bacc, bass_utils
from concourse.tile import TileContext

F32 = mybir.dt.float32
F32R = mybir.dt.float32r

S = 8          # samples per core
N = 512
ML = 3
KK = 7
NCORES = 8

TBL = 255                        # elements per slot table
BANDW = 128
NSLOTW = 42                      # 21 S1 + 21 S2
SLOT_S1 = 0
SLOT_S2 = 21
BFW = NSLOTW * BANDW             # 5376
TBLA_TOTAL = S * 3 * TBL         # 6120
TBLW_TOTAL = S * NSLOTW * TBL    # 85680

# Ax chunks: (o0, M); input x rows [o0-1, o0+127)
AX_CH = [(0, 126), (126, 126), (252, 126), (378, 126), (504, 8)]
# stage chunks: (o0, M); 122-row output chunks
S7_CH = [(0, 122), (122, 122), (244, 122), (366, 122), (488, 24)]
# rhs tile row starts (halo 3): tile k holds rows [RS[k], RS[k]+128)
RS = [-3, 119, 241, 363, 485]
# stage2 identity-add pairs: chunk j' -> [(xt tile j, shift, clip)]
ADD_PAIRS = [
    [(0, 1, False)],
    [(0, 123, True), (1, -3, False)],
    [(1, 119, True), (2, -7, False)],
    [(2, 115, True), (3, -11, False)],
    [(3, 111, True), (4, -15, False)],
]


def _sub_ap(base_ap, pattern, offset):
    """Custom access-pattern view: list of [step, count] pairs + elem offset."""
    a = base_ap.copy()
    v = a.ap
    v.clear()
    for p in pattern:
        v.append(list(p))
    a.offset = base_ap.offset + offset
    return a


def _overlaps_asc(o0, M):
    out = []
    for k, rs in enumerate(RS):
        lo = max(o0, rs, 0)
        hi = min(o0 + M, rs + 128, N)
        if hi > lo:
            out.append((k, lo - rs, lo - o0, hi - lo))
    return out


def _overlaps_desc(o0, M, qoff):
    out = []
    for k, rs in enumerate(RS):
        ck = rs + 127
        lo = max(o0, rs, 0)
        hi = min(o0 + M, rs + 128, N)
        if hi > lo:
            p0 = ck - (hi - 1)
            q0 = o0 + 121 - (hi - 1) - qoff
            out.append((k, p0, q0, hi - lo))
    return out


def build_kernel(nc):
    x = nc.dram_tensor("x", [S, N, N], F32, kind="ExternalInput").ap()
    f = nc.dram_tensor("f", [S, N, N], F32, kind="ExternalInput").ap()
    ka = nc.dram_tensor("kernelA", [S, 9], F32, kind="ExternalInput").ap()
    kaf = nc.dram_tensor("kernelA_flip", [S, 9], F32,
                         kind="ExternalInput").ap()
    fc_w1 = [nc.dram_tensor(f"fc{i}_w1", [100, 9], F32, kind="ExternalInput").ap()
             for i in (1, 2)]
    fc_b1 = [nc.dram_tensor(f"fc{i}_b1", [100], F32, kind="ExternalInput").ap()
             for i in (1, 2)]
    fc_w2 = [nc.dram_tensor(f"fc{i}_w2", [147, 100], F32, kind="ExternalInput").ap()
             for i in (1, 2)]
    fc_b2 = [nc.dram_tensor(f"fc{i}_b2", [147], F32, kind="ExternalInput").ap()
             for i in (1, 2)]
    out = nc.dram_tensor("out", [S, N, N], F32, kind="ExternalOutput").ap()

    with TileContext(nc) as tc:
        with (
            tc.tile_pool(name="dram", bufs=1, space="DRAM") as dpool,
            tc.tile_pool(name="const", bufs=1) as cpool,
            tc.tile_pool(name="mlp", bufs=1) as mpool,
            tc.tile_pool(name="bands", bufs=2) as bpool,
            tc.tile_pool(name="banda", bufs=2) as bapool,
            tc.tile_pool(name="rt", bufs=2) as rtpool,
            tc.tile_pool(name="tp", bufs=2) as tppool,
            tc.tile_pool(name="xa", bufs=3) as xa_pool,
            tc.tile_pool(name="fr", bufs=3) as fr_pool,
            tc.tile_pool(name="stout", bufs=2) as st_pool,
            tc.tile_pool(name="tm3", bufs=2) as tm3_pool,
            tc.tile_pool(name="psA", bufs=2, space="PSUM") as psA,
            tc.tile_pool(name="ps1", bufs=3, space="PSUM") as ps1,
            tc.tile_pool(name="ps2", bufs=2, space="PSUM") as ps2,
            tc.tile_pool(name="psx", bufs=1, space="PSUM") as psx,
        ):
            tablesA = dpool.tile([TBLA_TOTAL], F32)
            tablesW = dpool.tile([TBLW_TOTAL], F32)

            # ---- constants
            ident = cpool.tile([128, 128], F32)
            nc.gpsimd.memset(ident, 0.0)
            nc.gpsimd.affine_select(
                out=ident, in_=ident, compare_op=mybir.AluOpType.not_equal,
                fill=1.0, base=0, pattern=[[-1, 128]], channel_multiplier=1)
            # anti-diagonal reversal Rev[k,p] = d(k+p=127)
            rev_f = cpool.tile([128, 128], F32)
            nc.gpsimd.memset(rev_f, 0.0)
            nc.gpsimd.affine_select(
                out=rev_f, in_=rev_f, compare_op=mybir.AluOpType.not_equal,
                fill=1.0, base=-127, pattern=[[1, 128]], channel_multiplier=1)
            rev = cpool.tile([128, 128], F32R)
            nc.scalar.copy(rev, rev_f)   # round to f32r for the PE
            # double identity D[p, c] = d(p = c - 128), c in [128, 256)
            did_f = cpool.tile([128, 384], F32)
            nc.gpsimd.memset(did_f, 0.0)
            nc.gpsimd.affine_select(
                out=did_f, in_=did_f, compare_op=mybir.AluOpType.not_equal,
                fill=1.0, base=128, pattern=[[-1, 384]], channel_multiplier=1)
            d_full = cpool.tile([128, 384], F32R)
            nc.scalar.copy(d_full, did_f)
            d_clip = cpool.tile([128, 384], F32R)
            nc.scalar.copy(d_clip, did_f)
            # zero partitions 126,127 of the diagonal = zero cols 254,255
            nc.vector.memset(d_clip.bitcast(F32)[:, 254:256], 0.0)

            # ---- zero-fill tables (6120 = 8*765; 85680 = 112*765)
            ztA = tm3_pool.tile([8, 765], F32, name="ztA", tag="tm3")
            nc.vector.memset(ztA, 0.0)
            nc.gpsimd.dma_start(_sub_ap(tablesA, [[765, 8], [1, 765]], 0), ztA)
            ztW = tm3_pool.tile([112, 765], F32, name="ztW", tag="tm3")
            nc.vector.memset(ztW, 0.0)
            nc.gpsimd.dma_start(
                _sub_ap(tablesW, [[765, 112], [1, 765]], 0), ztW)

            # ---- A tables from host-flipped kernelA (no MLP dependency):
            # T_A[(s*3+kx)*255 + 125 + ky'] = A_flip[ky', kx]
            vTf = mpool.tile([9, S], F32, name="vTf")
            nc.sync.dma_start(vTf, kaf.rearrange("s k -> k s"))
            for kyf in range(3):
                nc.gpsimd.dma_start(
                    _sub_ap(tablesA, [[TBL, 3], [3 * TBL, S]], 125 + kyf),
                    vTf[3 * kyf:3 * kyf + 3, :])

            # ---------------- MLP + weight staging ----------------
            vT = mpool.tile([9, S], F32)
            nc.sync.dma_start(vT, ka.rearrange("s k -> k s"))

            w_sb = {}  # (layer i, map m) -> [49, S] conv weights
            for i in range(2):
                w1n = mpool.tile([100, 9], F32, name=f"w1n{i}")
                nc.sync.dma_start(w1n, fc_w1[i])
                W1T = mpool.tile([9, 100], F32, name=f"W1T{i}")
                t1 = psx.tile([9, 100], F32, name=f"t1_{i}", tag="aux")
                nc.tensor.transpose(t1, w1n, ident[:100, :100])
                nc.vector.tensor_copy(W1T, t1)

                b1 = mpool.tile([100, 1], F32, name=f"b1_{i}")
                nc.sync.dma_start(b1, fc_b1[i].unsqueeze(1))

                w2n_a = mpool.tile([128, 100], F32, name=f"w2na{i}")
                nc.sync.dma_start(w2n_a, fc_w2[i][0:128, :])
                w2n_b = mpool.tile([19, 100], F32, name=f"w2nb{i}")
                nc.sync.dma_start(w2n_b, fc_w2[i][128:147, :])
                W2T = mpool.tile([100, 147], F32, name=f"W2T{i}")
                tr_a = psx.tile([100, 128], F32, name=f"tra{i}", tag="aux")
                nc.tensor.transpose(tr_a, w2n_a, ident)
                nc.vector.tensor_copy(W2T[:, 0:128], tr_a)
                tr_b = psx.tile([100, 19], F32, name=f"trb{i}", tag="aux")
                nc.tensor.transpose(tr_b, w2n_b, ident[:19, :19])
                nc.vector.tensor_copy(W2T[:, 128:147], tr_b)

                h_pre = psx.tile([100, S], F32, name=f"hpre{i}", tag="aux")
                nc.tensor.matmul(h_pre, W1T, vT, start=True, stop=True)
                h = mpool.tile([100, S], F32, name=f"h{i}")
                nc.scalar.activation(
                    h, h_pre, mybir.ActivationFunctionType.Gelu, bias=b1)

                for m in range(ML):
                    b2m = mpool.tile([49, 1], F32, name=f"b2_{i}_{m}")
                    nc.sync.dma_start(
                        b2m, fc_b2[i][49 * m:49 * m + 49].unsqueeze(1))
                    wp = psx.tile([49, S], F32, name=f"wp{i}{m}", tag="aux")
                    nc.tensor.matmul(wp, W2T[:, 49 * m:49 * m + 49], h,
                                     start=True, stop=True)
                    wsb = mpool.tile([49, S], F32, name=f"w_{i}_{m}")
                    nc.scalar.activation(
                        wsb, wp, mybir.ActivationFunctionType.Identity,
                        bias=b2m)
                    w_sb[(i, m)] = wsb

            # ---- batched scatters (ascending everywhere; fc2 rows are
            # host-flipped so stage-2 uses the same +ky layout):
            # T_W[(s*42 + base + m*7 + kx)*255 + 121 + ky] = w[ky, kx]
            for i, base in ((0, SLOT_S1), (1, SLOT_S2)):
                for m in range(ML):
                    for ky in range(KK):
                        nc.gpsimd.dma_start(
                            _sub_ap(tablesW,
                                    [[TBL, KK], [NSLOTW * TBL, S]],
                                    (base + m * KK) * TBL + 121 + ky),
                            w_sb[(i, m)][KK * ky:KK * ky + KK, :])

            # ---------------- main per-sample loop (software-pipelined) ----
            bb_t, ba_t, rt_t, tp_t, xt_t = {}, {}, {}, {}, {}

            def band(s, slot, m0, m1):
                b = bb_t[s]
                return b[:, slot * BANDW + m0:slot * BANDW + m1]

            def emit_band_load(s):
                baw = bapool.tile([128, 3 * BANDW], F32R, name=f"baw{s}",
                                  tag="baw")
                nc.sync.dma_start(
                    _sub_ap(baw, [[3 * BANDW, 128], [BANDW, 3], [1, BANDW]],
                            0),
                    _sub_ap(tablesA.bitcast(F32R),
                            [[1, 128], [TBL, 3], [1, BANDW]], s * 3 * TBL))
                ba_t[s] = [baw]      # [0]=window, rev appends reversed
                bb = bpool.tile([128, BFW], F32R, name=f"bb{s}", tag="bb")
                bb_t[s] = bb
                nc.sync.dma_start(
                    _sub_ap(bb, [[BFW, 128], [BANDW, 21], [1, BANDW]], 0),
                    _sub_ap(tablesW.bitcast(F32R),
                            [[1, 128], [TBL, 21], [1, BANDW]],
                            s * NSLOTW * TBL))
                nc.scalar.dma_start(
                    _sub_ap(bb, [[BFW, 128], [BANDW, 21], [1, BANDW]],
                            21 * BANDW),
                    _sub_ap(tablesW.bitcast(F32R),
                            [[1, 128], [TBL, 21], [1, BANDW]],
                            (s * NSLOTW + 21) * TBL))

            def emit_x_load(s):
                xts = []
                for j, (o0, M) in enumerate(AX_CH):
                    row_start = o0 - 1
                    xt = xa_pool.tile([128, N + 2], F32R,
                                      name=f"xt{s}_{j}", tag=f"xa{j}")
                    xts.append(xt)
                    if row_start + 128 > N:          # bottom: ones pad
                        nc.gpsimd.memset(xt.bitcast(F32), 1.0)
                        nd = N - row_start
                        nc.sync.dma_start(
                            xt[0:nd, 1:N + 1],
                            x.bitcast(F32R)[s, row_start:N, :])
                        nc.gpsimd.memset(xt[0:nd, 0:1].bitcast(F32), 0.0)
                    else:
                        lo = max(0, row_start)
                        p0 = lo - row_start
                        if p0 > 0:
                            nc.gpsimd.memset(xt[0:p0, :].bitcast(F32), 0.0)
                        nc.sync.dma_start(
                            xt[p0:128, 1:N + 1],
                            x.bitcast(F32R)[s, lo:row_start + 128, :])
                        nc.gpsimd.memset(xt[:, 0:1].bitcast(F32), 0.0)
                        nc.gpsimd.memset(xt[:, N + 1:N + 2].bitcast(F32), 1.0)
                xt_t[s] = xts

            def emit_rev(s):
                baw = ba_t[s][0]
                ba = bapool.tile([128, 3 * BANDW], F32R, name=f"ba{s}",
                                 tag="ba")
                pr = psx.tile([128, 3 * BANDW], F32, name=f"pr{s}", tag="aux")
                nc.tensor.matmul(pr, rev, baw, start=True, stop=True)
                nc.scalar.copy(ba, pr)
                ba_t[s].append(ba)

            def emit_rhs_tiles(s):
                rt, tp = [], []
                for k in range(5):
                    t = rtpool.tile([128, N + 6], F32R, name=f"rt{s}_{k}",
                                    tag=f"rt{k}")
                    rt.append(t)
                    if s < 2:
                        nc.gpsimd.memset(t.bitcast(F32), 0.0)
                    t2 = tppool.tile([128, ML * (N + 6)], F32R,
                                     name=f"tp{s}_{k}", tag=f"tp{k}")
                    tp.append(t2)
                    if s < 2:
                        nc.gpsimd.memset(t2.bitcast(F32), 0.0)
                rt_t[s], tp_t[s] = rt, tp

            def emit_ax(s):
                ba, rt = ba_t[s][1], rt_t[s]
                for j, (o0, M) in enumerate(AX_CH):
                    xt = xt_t[s][j]
                    ft = fr_pool.tile([126, N], F32, name=f"ft{s}_{j}",
                                      tag="f")
                    nc.scalar.dma_start(ft[:M, :], f[s, o0:o0 + M, :])
                    ps = psA.tile([M, N], F32, name=f"psA{s}_{j}", tag="ax")
                    for kx in range(3):
                        nc.tensor.matmul(ps, ba[:, kx * BANDW:kx * BANDW + M],
                                         xt[:, kx:kx + N],
                                         start=(kx == 0), stop=(kx == 2))
                    rf = fr_pool.tile([126, N], F32, name=f"rf{s}_{j}",
                                      tag="rf")
                    nc.vector.tensor_sub(rf[:M, :], ft[:M, :], ps[:M, :])
                    for (k, p0, q0, n) in _overlaps_asc(o0, M):
                        eng = nc.sync if n > 32 else nc.gpsimd
                        eng.dma_start(
                            _sub_ap(rt[k], [[N + 6, n], [1, N]],
                                    p0 * (N + 6) + 3),
                            _sub_ap(rf.bitcast(F32R), [[N, n], [1, N]],
                                    q0 * N))

            def emit_stage1(s):
                rt, tp = rt_t[s], tp_t[s]
                for j, (o0, M) in enumerate(S7_CH):
                    qoff = 98 if M < 122 else 0   # lhsT col slice for c4
                    tm3 = tm3_pool.tile([122, ML * N], F32,
                                        name=f"tm3_{s}_{j}", tag="tm3")
                    for m in range(ML):
                        ps_ = ps1.tile([122, N], F32, name=f"ps1_{s}_{j}_{m}",
                                       tag="s1")
                        for kx in range(KK):
                            nc.tensor.matmul(
                                ps_[:M, :],
                                band(s, SLOT_S1 + m * KK + kx, qoff,
                                     qoff + M),
                                rt[j][:, kx:kx + N],
                                start=(kx == 0), stop=(kx == KK - 1))
                        nc.vector.tensor_copy(
                            tm3[:M, m * N:(m + 1) * N], ps_[:M, :])
                    for (k, p0, q0, n) in _overlaps_desc(o0, M, qoff):
                        eng = (nc.sync if j % 2 == 0 else
                               nc.scalar) if n > 32 else nc.gpsimd
                        eng.dma_start(
                            _sub_ap(tp[k],
                                    [[ML * (N + 6), n], [N + 6, ML], [1, N]],
                                    p0 * ML * (N + 6) + 3),
                            _sub_ap(tm3.bitcast(F32R),
                                    [[ML * N, n], [N, ML], [1, N]],
                                    q0 * ML * N))

            def emit_stage2(s):
                tp, xts = tp_t[s], xt_t[s]
                for j, (o0, M) in enumerate(S7_CH):
                    pg = ps2.tile([122, N], F32, name=f"ps2_{s}_{j}", tag="s2")
                    nmm = 21 + len(ADD_PAIRS[j])
                    idx = 0
                    for m in range(ML):
                        for kx in range(KK):
                            nc.tensor.matmul(
                                pg[:M, :],
                                band(s, SLOT_S2 + m * KK + kx, 0, M),
                                tp[j][:, m * (N + 6) + kx:
                                      m * (N + 6) + kx + N],
                                start=(idx == 0), stop=(idx == nmm - 1))
                            idx += 1
                    # out = x + G2 on the PE via double-identity bands
                    for (jx, shift, clip) in ADD_PAIRS[j]:
                        dd = d_clip if clip else d_full
                        nc.tensor.matmul(
                            pg[:M, :], dd[:, 128 + shift:128 + shift + M],
                            xts[jx][:, 1:N + 1],
                            start=False, stop=(idx == nmm - 1))
                        idx += 1
                    ob = st_pool.tile([122, N], F32, name=f"ob{s}_{j}",
                                      tag="ob")
                    nc.vector.tensor_copy(ob[:M, :], pg[:M, :])
                    nc.scalar.dma_start(out[s, o0:o0 + M, :], ob[:M, :])
                del bb_t[s], ba_t[s], rt_t[s], tp_t[s], xt_t[s]

            # prologue
            emit_band_load(0)
            emit_x_load(0)
            for s in range(S):
                emit_rev(s)
                emit_rhs_tiles(s)
                emit_ax(s)
                # prefetch next iteration inputs
                if s + 1 < S:
                    emit_band_load(s + 1)
                    emit_x_load(s + 1)
                if s >= 1:
                    emit_stage2(s - 1)
                emit_stage1(s)
            emit_stage2(S - 1)
    return nc


_CACHED = None


def _get_nc():
    global _CACHED
    if _CACHED is None:
        nc = bacc.Bacc("TRN2", debug=False, enable_asserts=False,
                       num_devices=NCORES)
        build_kernel(nc)
        nc.compile()
        _CACHED = nc
    return _CACHED


def make_in_maps(x, f, kernelA, fc1_w1, fc1_b1, fc1_w2, fc1_b2,
                 fc2_w1, fc2_b1, fc2_w2, fc2_b2):
    # stage-2 ky flip lives host-side: reorder fc2_w2/fc2_b2 output rows
    # (147 = 3 maps x 7 ky x 7 kx) so the on-device scatter is ascending.
    w2f = np.ascontiguousarray(
        np.asarray(fc2_w2, np.float32).reshape(ML, KK, KK, 100)[:, ::-1]
        .reshape(ML * KK * KK, 100))
    b2f = np.ascontiguousarray(
        np.asarray(fc2_b2, np.float32).reshape(ML, KK, KK)[:, ::-1]
        .reshape(ML * KK * KK))
    shared = {
        "fc1_w1": np.ascontiguousarray(fc1_w1, np.float32),
        "fc1_b1": np.ascontiguousarray(fc1_b1, np.float32),
        "fc1_w2": np.ascontiguousarray(fc1_w2, np.float32),
        "fc1_b2": np.ascontiguousarray(fc1_b2, np.float32),
        "fc2_w1": np.ascontiguousarray(fc2_w1, np.float32),
        "fc2_b1": np.ascontiguousarray(fc2_b1, np.float32),
        "fc2_w2": w2f,
        "fc2_b2": b2f,
    }
    in_maps = []
    for c in range(NCORES):
        sl = slice(S * c, S * (c + 1))
        kac = np.ascontiguousarray(
            kernelA[sl, 0].reshape(S, 9), np.float32)
        kaflip = np.ascontiguousarray(
            kac.reshape(S, 3, 3)[:, ::-1].reshape(S, 9))
        in_maps.append({
            "x": np.ascontiguousarray(x[sl, 0], np.float32),
            "f": np.ascontiguousarray(f[sl, 0], np.float32),
            "kernelA": kac,
            "kernelA_flip": kaflip,
            **shared,
        })
    return in_maps


def kernel(x, f, kernelA, fc1_w1, fc1_b1, fc1_w2, fc1_b2,
           fc2_w1, fc2_b1, fc2_w2, fc2_b2):
    x = np.asarray(x)
    nc = _get_nc()
    in_maps = make_in_maps(x, f, kernelA, fc1_w1, fc1_b1, fc1_w2, fc1_b2,
                           fc2_w1, fc2_b1, fc2_w2, fc2_b2)
    res = bass_utils.run_bass_kernel_spmd(
        nc, in_maps, core_ids=list(range(NCORES)))
    outs = [res.results[c]["out"] for c in range(NCORES)]
    full = np.concatenate(outs, axis=0).reshape(64, 1, N, N).astype(np.float32)
    return full


# revision 17
# speedup vs baseline: 1.1076x; 1.0354x over previous
"""MetaConvSmoother Trainium2 kernel (Bass/Tile), data-parallel over 8 NeuronCores.

v4: SBUF-resident pipeline, software-pipelined across samples.

Per core (8 samples):
  - hypernet MLPs (9 -> 100 -> 147, exact gelu) on PE + ACT
  - per-sample conv kernels staged as zero-padded tables in DRAM;
    bands loaded as overlapping windows Bf[p, m] = T[p + m] with
    all-positive strides.  fc2_w2/fc2_b2 arrive ROW-FLIPPED in ky
    (host-side) so every table scatter is an ascending batched DMA;
    kernelA_flip likewise feeds the A tables.
  - parity trick: window band on ASCENDING rhs -> DESCENDING output and
    vice versa.  Chain: Ax (PE-reversed A band, asc->asc) -> r asc ->
    stage1 (window, asc->desc) -> tmp desc -> stage2 (window,
    desc->asc) -> out.  Only the 3-slot A band is PE-reversed.
  - r and tmp live in SBUF: PSUM chunks go through base-0 staging
    tiles (DVE) then SBUF->SBUF DMA scatters into halo-tiled rhs
    tiles (compute engines cannot address partitions off 32-alignment,
    DMA can).
  - out = x + G2 accumulated ON THE PE: const double-identity bands
    (D_full / D_clip) add the x tiles into the stage-2 PSUM, removing
    the x2 reload and its DVE dependency chain.
  - per-iteration PE order: rev(s), Ax(s), stage2(s-1), stage1(s) so
    scatter latencies hide behind ready matmul work.
"""
import numpy as np

import concourse.bass as bass
import concourse.mybir as mybir
from concourse import bacc, bass_utils
from concourse.tile import TileContext

F32 = mybir.dt.float32
F32R = mybir.dt.float32r

S = 8          # samples per core
N = 512
ML = 3
KK = 7
NCORES = 8

TBL = 255                        # elements per slot table
BANDW = 128
NSLOTW = 42                      # 21 S1 + 21 S2
SLOT_S1 = 0
SLOT_S2 = 21
BFW = NSLOTW * BANDW             # 5376
TBLA_TOTAL = S * 3 * TBL         # 6120
TBLW_TOTAL = S * NSLOTW * TBL    # 85680

# Ax chunks: (o0, M); input x rows [o0-1, o0+127)
AX_CH = [(0, 126), (126, 126), (252, 126), (378, 126), (504, 8)]
# stage chunks: (o0, M); 122-row output chunks
S7_CH = [(0, 122), (122, 122), (244, 122), (366, 122), (488, 24)]
# rhs tile row starts (halo 3): tile k holds rows [RS[k], RS[k]+128)
RS = [-3, 119, 241, 363, 485]
# stage2 identity-add pairs: chunk j' -> [(xt tile j, shift, clip)]
ADD_PAIRS = [
    [(0, 1, False)],
    [(0, 123, True), (1, -3, False)],
    [(1, 119, True), (2, -7, False)],
    [(2, 115, True), (3, -11, False)],
    [(3, 111, True), (4, -15, False)],
]


def _sub_ap(base_ap, pattern, offset):
    """Custom access-pattern view: list of [step, count] pairs + elem offset."""
    a = base_ap.copy()
    v = a.ap
    v.clear()
    for p in pattern:
        v.append(list(p))
    a.offset = base_ap.offset + offset
    return a


def _overlaps_asc(o0, M):
    out = []
    for k, rs in enumerate(RS):
        lo = max(o0, rs, 0)
        hi = min(o0 + M, rs + 128, N)
        if hi > lo:
            out.append((k, lo - rs, lo - o0, hi - lo))
    return out


def _overlaps_desc(o0, M, qoff):
    out = []
    for k, rs in enumerate(RS):
        ck = rs + 127
        lo = max(o0, rs, 0)
        hi = min(o0 + M, rs + 128, N)
        if hi > lo:
            p0 = ck - (hi - 1)
            q0 = o0 + 121 - (hi - 1) - qoff
            out.append((k, p0, q0, hi - lo))
    return out


def build_kernel(nc):
    x = nc.dram_tensor("x", [S, N, N], F32, kind="ExternalInput").ap()
    f = nc.dram_tensor("f", [S, N, N], F32, kind="ExternalInput").ap()
    ka = nc.dram_tensor("kernelA", [S, 9], F32, kind="ExternalInput").ap()
    kaf = nc.dram_tensor("kernelA_flip", [S, 9], F32,
                         kind="ExternalInput").ap()
    fc_w1 = [nc.dram_tensor(f"fc{i}_w1", [100, 9], F32, kind="ExternalInput").ap()
             for i in (1, 2)]
    fc_b1 = [nc.dram_tensor(f"fc{i}_b1", [100], F32, kind="ExternalInput").ap()
             for i in (1, 2)]
    fc_w2 = [nc.dram_tensor(f"fc{i}_w2", [147, 100], F32, kind="ExternalInput").ap()
             for i in (1, 2)]
    fc_b2 = [nc.dram_tensor(f"fc{i}_b2", [147], F32, kind="ExternalInput").ap()
             for i in (1, 2)]
    out = nc.dram_tensor("out", [S, N, N], F32, kind="ExternalOutput").ap()

    with TileContext(nc) as tc:
        with (
            tc.tile_pool(name="dram", bufs=1, space="DRAM") as dpool,
            tc.tile_pool(name="const", bufs=1) as cpool,
            tc.tile_pool(name="mlp", bufs=1) as mpool,
            tc.tile_pool(name="bands", bufs=2) as bpool,
            tc.tile_pool(name="banda", bufs=2) as bapool,
            tc.tile_pool(name="rt", bufs=2) as rtpool,
            tc.tile_pool(name="tp", bufs=2) as tppool,
            tc.tile_pool(name="xa", bufs=2) as xa_pool,
            tc.tile_pool(name="fr", bufs=3) as fr_pool,
            tc.tile_pool(name="stout", bufs=3) as st_pool,
            tc.tile_pool(name="tm3", bufs=4) as tm3_pool,
            tc.tile_pool(name="psA", bufs=1, space="PSUM") as psA,
            tc.tile_pool(name="ps1", bufs=4, space="PSUM") as ps1,
            tc.tile_pool(name="ps2", bufs=2, space="PSUM") as ps2,
            tc.tile_pool(name="psx", bufs=1, space="PSUM") as psx,
        ):
            tablesA = dpool.tile([TBLA_TOTAL], F32)
            tablesW = dpool.tile([TBLW_TOTAL], F32)

            # ---- constants
            ident = cpool.tile([128, 128], F32)
            nc.gpsimd.memset(ident, 0.0)
            nc.gpsimd.affine_select(
                out=ident, in_=ident, compare_op=mybir.AluOpType.not_equal,
                fill=1.0, base=0, pattern=[[-1, 128]], channel_multiplier=1)
            # anti-diagonal reversal Rev[k,p] = d(k+p=127)
            rev_f = cpool.tile([128, 128], F32)
            nc.gpsimd.memset(rev_f, 0.0)
            nc.gpsimd.affine_select(
                out=rev_f, in_=rev_f, compare_op=mybir.AluOpType.not_equal,
                fill=1.0, base=-127, pattern=[[1, 128]], channel_multiplier=1)
            rev = cpool.tile([128, 128], F32R)
            nc.scalar.copy(rev, rev_f)   # round to f32r for the PE
            # double identity D[p, c] = d(p = c - 128), c in [128, 256)
            did_f = cpool.tile([128, 384], F32)
            nc.gpsimd.memset(did_f, 0.0)
            nc.gpsimd.affine_select(
                out=did_f, in_=did_f, compare_op=mybir.AluOpType.not_equal,
                fill=1.0, base=128, pattern=[[-1, 384]], channel_multiplier=1)
            d_full = cpool.tile([128, 384], F32R)
            nc.scalar.copy(d_full, did_f)
            d_clip = cpool.tile([128, 384], F32R)
            nc.scalar.copy(d_clip, did_f)
            # zero partitions 126,127 of the diagonal = zero cols 254,255
            nc.vector.memset(d_clip.bitcast(F32)[:, 254:256], 0.0)

            # ---- zero-fill tables (6120 = 8*765; 85680 = 112*765)
            ztA = tm3_pool.tile([8, 765], F32, name="ztA", tag="tm3")
            nc.vector.memset(ztA, 0.0)
            nc.gpsimd.dma_start(_sub_ap(tablesA, [[765, 8], [1, 765]], 0), ztA)
            ztW = tm3_pool.tile([112, 765], F32, name="ztW", tag="tm3")
            nc.vector.memset(ztW, 0.0)
            nc.gpsimd.dma_start(
                _sub_ap(tablesW, [[765, 112], [1, 765]], 0), ztW)

            # ---- A tables from host-flipped kernelA (no MLP dependency):
            # T_A[(s*3+kx)*255 + 125 + ky'] = A_flip[ky', kx]
            vTf = mpool.tile([9, S], F32, name="vTf")
            nc.sync.dma_start(vTf, kaf.rearrange("s k -> k s"))
            for kyf in range(3):
                nc.gpsimd.dma_start(
                    _sub_ap(tablesA, [[TBL, 3], [3 * TBL, S]], 125 + kyf),
                    vTf[3 * kyf:3 * kyf + 3, :])

            # ---------------- MLP + weight staging ----------------
            vT = mpool.tile([9, S], F32)
            nc.sync.dma_start(vT, ka.rearrange("s k -> k s"))

            w_sb = {}  # (layer i, map m) -> [49, S] conv weights
            for i in range(2):
                w1n = mpool.tile([100, 9], F32, name=f"w1n{i}")
                nc.sync.dma_start(w1n, fc_w1[i])
                W1T = mpool.tile([9, 100], F32, name=f"W1T{i}")
                t1 = psx.tile([9, 100], F32, name=f"t1_{i}", tag="aux")
                nc.tensor.transpose(t1, w1n, ident[:100, :100])
                nc.vector.tensor_copy(W1T, t1)

                b1 = mpool.tile([100, 1], F32, name=f"b1_{i}")
                nc.sync.dma_start(b1, fc_b1[i].unsqueeze(1))

                w2n_a = mpool.tile([128, 100], F32, name=f"w2na{i}")
                nc.sync.dma_start(w2n_a, fc_w2[i][0:128, :])
                w2n_b = mpool.tile([19, 100], F32, name=f"w2nb{i}")
                nc.sync.dma_start(w2n_b, fc_w2[i][128:147, :])
                W2T = mpool.tile([100, 147], F32, name=f"W2T{i}")
                tr_a = psx.tile([100, 128], F32, name=f"tra{i}", tag="aux")
                nc.tensor.transpose(tr_a, w2n_a, ident)
                nc.vector.tensor_copy(W2T[:, 0:128], tr_a)
                tr_b = psx.tile([100, 19], F32, name=f"trb{i}", tag="aux")
                nc.tensor.transpose(tr_b, w2n_b, ident[:19, :19])
                nc.vector.tensor_copy(W2T[:, 128:147], tr_b)

                h_pre = psx.tile([100, S], F32, name=f"hpre{i}", tag="aux")
                nc.tensor.matmul(h_pre, W1T, vT, start=True, stop=True)
                h = mpool.tile([100, S], F32, name=f"h{i}")
                nc.scalar.activation(
                    h, h_pre, mybir.ActivationFunctionType.Gelu, bias=b1)

                for m in range(ML):
                    b2m = mpool.tile([49, 1], F32, name=f"b2_{i}_{m}")
                    nc.sync.dma_start(
                        b2m, fc_b2[i][49 * m:49 * m + 49].unsqueeze(1))
                    wp = psx.tile([49, S], F32, name=f"wp{i}{m}", tag="aux")
                    nc.tensor.matmul(wp, W2T[:, 49 * m:49 * m + 49], h,
                                     start=True, stop=True)
                    wsb = mpool.tile([49, S], F32, name=f"w_{i}_{m}")
                    nc.scalar.activation(
                        wsb, wp, mybir.ActivationFunctionType.Identity,
                        bias=b2m)
                    w_sb[(i, m)] = wsb

            # ---- batched scatters (ascending everywhere; fc2 rows are
            # host-flipped so stage-2 uses the same +ky layout):
            # T_W[(s*42 + base + m*7 + kx)*255 + 121 + ky] = w[ky, kx]
            for i, base in ((0, SLOT_S1), (1, SLOT_S2)):
                for m in range(ML):
                    for ky in range(KK):
                        nc.gpsimd.dma_start(
                            _sub_ap(tablesW,
                                    [[TBL, KK], [NSLOTW * TBL, S]],
                                    (base + m * KK) * TBL + 121 + ky),
                            w_sb[(i, m)][KK * ky:KK * ky + KK, :])

            # ---------------- main per-sample loop (software-pipelined) ----
            bb_t, ba_t, rt_t, tp_t, xt_t = {}, {}, {}, {}, {}

            def band(s, slot, m0, m1):
                b = bb_t[s]
                return b[:, slot * BANDW + m0:slot * BANDW + m1]

            def emit_band_load(s):
                baw = bapool.tile([128, 3 * BANDW], F32R, name=f"baw{s}",
                                  tag="baw")
                nc.sync.dma_start(
                    _sub_ap(baw, [[3 * BANDW, 128], [BANDW, 3], [1, BANDW]],
                            0),
                    _sub_ap(tablesA.bitcast(F32R),
                            [[1, 128], [TBL, 3], [1, BANDW]], s * 3 * TBL))
                ba_t[s] = [baw]      # [0]=window, rev appends reversed
                bb = bpool.tile([128, BFW], F32R, name=f"bb{s}", tag="bb")
                bb_t[s] = bb
                nc.sync.dma_start(
                    _sub_ap(bb, [[BFW, 128], [BANDW, 21], [1, BANDW]], 0),
                    _sub_ap(tablesW.bitcast(F32R),
                            [[1, 128], [TBL, 21], [1, BANDW]],
                            s * NSLOTW * TBL))
                nc.scalar.dma_start(
                    _sub_ap(bb, [[BFW, 128], [BANDW, 21], [1, BANDW]],
                            21 * BANDW),
                    _sub_ap(tablesW.bitcast(F32R),
                            [[1, 128], [TBL, 21], [1, BANDW]],
                            (s * NSLOTW + 21) * TBL))

            def emit_x_load(s):
                xts = []
                for j, (o0, M) in enumerate(AX_CH):
                    row_start = o0 - 1
                    xt = xa_pool.tile([128, N + 2], F32R,
                                      name=f"xt{s}_{j}", tag=f"xa{j}")
                    xts.append(xt)
                    if row_start + 128 > N:          # bottom: ones pad
                        nc.gpsimd.memset(xt.bitcast(F32), 1.0)
                        nd = N - row_start
                        nc.sync.dma_start(
                            xt[0:nd, 1:N + 1],
                            x.bitcast(F32R)[s, row_start:N, :])
                        nc.gpsimd.memset(xt[0:nd, 0:1].bitcast(F32), 0.0)
                    else:
                        lo = max(0, row_start)
                        p0 = lo - row_start
                        if p0 > 0:
                            nc.gpsimd.memset(xt[0:p0, :].bitcast(F32), 0.0)
                        nc.sync.dma_start(
                            xt[p0:128, 1:N + 1],
                            x.bitcast(F32R)[s, lo:row_start + 128, :])
                        nc.gpsimd.memset(xt[:, 0:1].bitcast(F32), 0.0)
                        nc.gpsimd.memset(xt[:, N + 1:N + 2].bitcast(F32), 1.0)
                xt_t[s] = xts

            def emit_rev(s):
                baw = ba_t[s][0]
                ba = bapool.tile([128, 3 * BANDW], F32R, name=f"ba{s}",
                                 tag="ba")
                pr = psx.tile([128, 3 * BANDW], F32, name=f"pr{s}", tag="aux")
                nc.tensor.matmul(pr, rev, baw, start=True, stop=True)
                nc.scalar.copy(ba, pr)
                ba_t[s].append(ba)

            def emit_rhs_tiles(s):
                rt, tp = [], []
                for k in range(5):
                    t = rtpool.tile([128, N + 6], F32R, name=f"rt{s}_{k}",
                                    tag=f"rt{k}")
                    rt.append(t)
                    if s < 2:
                        nc.gpsimd.memset(t.bitcast(F32), 0.0)
                    t2 = tppool.tile([128, ML * (N + 6)], F32R,
                                     name=f"tp{s}_{k}", tag=f"tp{k}")
                    tp.append(t2)
                    if s < 2:
                        nc.gpsimd.memset(t2.bitcast(F32), 0.0)
                rt_t[s], tp_t[s] = rt, tp

            def emit_ax(s):
                ba, rt = ba_t[s][1], rt_t[s]
                for j, (o0, M) in enumerate(AX_CH):
                    xt = xt_t[s][j]
                    ft = fr_pool.tile([126, N], F32, name=f"ft{s}_{j}",
                                      tag="f")
                    nc.scalar.dma_start(ft[:M, :], f[s, o0:o0 + M, :])
                    ps = psA.tile([M, N], F32, name=f"psA{s}_{j}", tag="ax")
                    for kx in range(3):
                        nc.tensor.matmul(ps, ba[:, kx * BANDW:kx * BANDW + M],
                                         xt[:, kx:kx + N],
                                         start=(kx == 0), stop=(kx == 2))
                    rf = fr_pool.tile([126, N], F32, name=f"rf{s}_{j}",
                                      tag="rf")
                    nc.vector.tensor_sub(rf[:M, :], ft[:M, :], ps[:M, :])
                    for (k, p0, q0, n) in _overlaps_asc(o0, M):
                        eng = nc.sync if j % 2 == 0 else nc.scalar
                        eng.dma_start(
                            _sub_ap(rt[k], [[N + 6, n], [1, N]],
                                    p0 * (N + 6) + 3),
                            _sub_ap(rf.bitcast(F32R), [[N, n], [1, N]],
                                    q0 * N))

            def emit_stage1(s):
                rt, tp = rt_t[s], tp_t[s]
                for j, (o0, M) in enumerate(S7_CH):
                    qoff = 98 if M < 122 else 0   # lhsT col slice for c4
                    tm3 = tm3_pool.tile([122, ML * N], F32,
                                        name=f"tm3_{s}_{j}", tag="tm3")
                    for m in range(ML):
                        ps_ = ps1.tile([122, N], F32, name=f"ps1_{s}_{j}_{m}",
                                       tag="s1")
                        for kx in range(KK):
                            nc.tensor.matmul(
                                ps_[:M, :],
                                band(s, SLOT_S1 + m * KK + kx, qoff,
                                     qoff + M),
                                rt[j][:, kx:kx + N],
                                start=(kx == 0), stop=(kx == KK - 1))
                        if m == 1:
                            nc.scalar.copy(
                                tm3[:M, m * N:(m + 1) * N], ps_[:M, :])
                        else:
                            nc.vector.tensor_copy(
                                tm3[:M, m * N:(m + 1) * N], ps_[:M, :])
                    for (k, p0, q0, n) in _overlaps_desc(o0, M, qoff):
                        eng = nc.sync if j % 2 == 0 else nc.scalar
                        eng.dma_start(
                            _sub_ap(tp[k],
                                    [[ML * (N + 6), n], [N + 6, ML], [1, N]],
                                    p0 * ML * (N + 6) + 3),
                            _sub_ap(tm3.bitcast(F32R),
                                    [[ML * N, n], [N, ML], [1, N]],
                                    q0 * ML * N))

            def emit_stage2(s):
                tp, xts = tp_t[s], xt_t[s]
                for j, (o0, M) in enumerate(S7_CH):
                    pg = ps2.tile([122, N], F32, name=f"ps2_{s}_{j}", tag="s2")
                    nmm = 21 + len(ADD_PAIRS[j])
                    idx = 0
                    for m in range(ML):
                        for kx in range(KK):
                            nc.tensor.matmul(
                                pg[:M, :],
                                band(s, SLOT_S2 + m * KK + kx, 0, M),
                                tp[j][:, m * (N + 6) + kx:
                                      m * (N + 6) + kx + N],
                                start=(idx == 0), stop=(idx == nmm - 1))
                            idx += 1
                    # out = x + G2 on the PE via double-identity bands
                    for (jx, shift, clip) in ADD_PAIRS[j]:
                        dd = d_clip if clip else d_full
                        nc.tensor.matmul(
                            pg[:M, :], dd[:, 128 + shift:128 + shift + M],
                            xts[jx][:, 1:N + 1],
                            start=False, stop=(idx == nmm - 1))
                        idx += 1
                    ob = st_pool.tile([122, N], F32, name=f"ob{s}_{j}",
                                      tag="ob")
                    nc.scalar.copy(ob[:M, :], pg[:M, :])
                    nc.scalar.dma_start(out[s, o0:o0 + M, :], ob[:M, :])
                del bb_t[s], ba_t[s], rt_t[s], tp_t[s], xt_t[s]

            # prologue
            emit_band_load(0)
            emit_x_load(0)
            for s in range(S):
                emit_rev(s)
                emit_rhs_tiles(s)
                emit_ax(s)
                # prefetch next iteration inputs
                if s + 1 < S:
                    emit_band_load(s + 1)
                    emit_x_load(s + 1)
                if s >= 1:
                    emit_stage2(s - 1)
                emit_stage1(s)
            emit_stage2(S - 1)
    return nc


_CACHED = None


def _get_nc():
    global _CACHED
    if _CACHED is None:
        nc = bacc.Bacc("TRN2", debug=False, enable_asserts=False,
                       num_devices=NCORES)
        build_kernel(nc)
        nc.compile()
        _CACHED = nc
    return _CACHED


def make_in_maps(x, f, kernelA, fc1_w1, fc1_b1, fc1_w2, fc1_b2,
                 fc2_w1, fc2_b1, fc2_w2, fc2_b2):
    # stage-2 ky flip lives host-side: reorder fc2_w2/fc2_b2 output rows
    # (147 = 3 maps x 7 ky x 7 kx) so the on-device scatter is ascending.
    w2f = np.ascontiguousarray(
        np.asarray(fc2_w2, np.float32).reshape(ML, KK, KK, 100)[:, ::-1]
        .reshape(ML * KK * KK, 100))
    b2f = np.ascontiguousarray(
        np.asarray(fc2_b2, np.float32).reshape(ML, KK, KK)[:, ::-1]
        .reshape(ML * KK * KK))
    shared = {
        "fc1_w1": np.ascontiguousarray(fc1_w1, np.float32),
        "fc1_b1": np.ascontiguousarray(fc1_b1, np.float32),
        "fc1_w2": np.ascontiguousarray(fc1_w2, np.float32),
        "fc1_b2": np.ascontiguousarray(fc1_b2, np.float32),
        "fc2_w1": np.ascontiguousarray(fc2_w1, np.float32),
        "fc2_b1": np.ascontiguousarray(fc2_b1, np.float32),
        "fc2_w2": w2f,
        "fc2_b2": b2f,
    }
    in_maps = []
    for c in range(NCORES):
        sl = slice(S * c, S * (c + 1))
        kac = np.ascontiguousarray(
            kernelA[sl, 0].reshape(S, 9), np.float32)
        kaflip = np.ascontiguousarray(
            kac.reshape(S, 3, 3)[:, ::-1].reshape(S, 9))
        in_maps.append({
            "x": np.ascontiguousarray(x[sl, 0], np.float32),
            "f": np.ascontiguousarray(f[sl, 0], np.float32),
            "kernelA": kac,
            "kernelA_flip": kaflip,
            **shared,
        })
    return in_maps


def kernel(x, f, kernelA, fc1_w1, fc1_b1, fc1_w2, fc1_b2,
           fc2_w1, fc2_b1, fc2_w2, fc2_b2):
    x = np.asarray(x)
    nc = _get_nc()
    in_maps = make_in_maps(x, f, kernelA, fc1_w1, fc1_b1, fc1_w2, fc1_b2,
                           fc2_w1, fc2_b1, fc2_w2, fc2_b2)
    res = bass_utils.run_bass_kernel_spmd(
        nc, in_maps, core_ids=list(range(NCORES)))
    outs = [res.results[c]["out"] for c in range(NCORES)]
    full = np.concatenate(outs, axis=0).reshape(64, 1, N, N).astype(np.float32)
    return full


# revision 18
# speedup vs baseline: 1.1113x; 1.0033x over previous
"""MetaConvSmoother Trainium2 kernel (Bass/Tile), data-parallel over 8 NeuronCores.

v4: SBUF-resident pipeline, software-pipelined across samples.

Per core (8 samples):
  - hypernet MLPs (9 -> 100 -> 147, exact gelu) on PE + ACT
  - per-sample conv kernels staged as zero-padded tables in DRAM;
    bands loaded as overlapping windows Bf[p, m] = T[p + m] with
    all-positive strides.  fc2_w2/fc2_b2 arrive ROW-FLIPPED in ky
    (host-side) so every table scatter is an ascending batched DMA;
    kernelA_flip likewise feeds the A tables.
  - parity trick: window band on ASCENDING rhs -> DESCENDING output and
    vice versa.  Chain: Ax (PE-reversed A band, asc->asc) -> r asc ->
    stage1 (window, asc->desc) -> tmp desc -> stage2 (window,
    desc->asc) -> out.  Only the 3-slot A band is PE-reversed.
  - r and tmp live in SBUF: PSUM chunks go through base-0 staging
    tiles (DVE) then SBUF->SBUF DMA scatters into halo-tiled rhs
    tiles (compute engines cannot address partitions off 32-alignment,
    DMA can).
  - out = x + G2 accumulated ON THE PE: const double-identity bands
    (D_full / D_clip) add the x tiles into the stage-2 PSUM, removing
    the x2 reload and its DVE dependency chain.
  - per-iteration PE order: rev(s), Ax(s), stage2(s-1), stage1(s) so
    scatter latencies hide behind ready matmul work.
"""
import numpy as np

import concourse.bass as bass
import concourse.mybir as mybir
from concourse import bacc, bass_utils
from concourse.tile import TileContext

F32 = mybir.dt.float32
F32R = mybir.dt.float32r

S = 8          # samples per core
N = 512
ML = 3
KK = 7
NCORES = 8

TBL = 255                        # elements per slot table
BANDW = 128
NSLOTW = 42                      # 21 S1 + 21 S2
SLOT_S1 = 0
SLOT_S2 = 21
BFW = NSLOTW * BANDW             # 5376
TBLA_TOTAL = S * 3 * TBL         # 6120
TBLW_TOTAL = S * NSLOTW * TBL    # 85680

# Ax chunks: (o0, M); input x rows [o0-1, o0+127)
AX_CH = [(0, 126), (126, 126), (252, 126), (378, 126), (504, 8)]
# stage chunks: (o0, M); 122-row output chunks
S7_CH = [(0, 122), (122, 122), (244, 122), (366, 122), (488, 24)]
# rhs tile row starts (halo 3): tile k holds rows [RS[k], RS[k]+128)
RS = [-3, 119, 241, 363, 485]
# stage2 identity-add pairs: chunk j' -> [(xt tile j, shift, clip)]
ADD_PAIRS = [
    [(0, 1, False)],
    [(0, 123, True), (1, -3, False)],
    [(1, 119, True), (2, -7, False)],
    [(2, 115, True), (3, -11, False)],
    [(3, 111, True), (4, -15, False)],
]


def _sub_ap(base_ap, pattern, offset):
    """Custom access-pattern view: list of [step, count] pairs + elem offset."""
    a = base_ap.copy()
    v = a.ap
    v.clear()
    for p in pattern:
        v.append(list(p))
    a.offset = base_ap.offset + offset
    return a


def _overlaps_asc(o0, M):
    out = []
    for k, rs in enumerate(RS):
        lo = max(o0, rs, 0)
        hi = min(o0 + M, rs + 128, N)
        if hi > lo:
            out.append((k, lo - rs, lo - o0, hi - lo))
    return out


def _overlaps_desc(o0, M, qoff):
    out = []
    for k, rs in enumerate(RS):
        ck = rs + 127
        lo = max(o0, rs, 0)
        hi = min(o0 + M, rs + 128, N)
        if hi > lo:
            p0 = ck - (hi - 1)
            q0 = o0 + 121 - (hi - 1) - qoff
            out.append((k, p0, q0, hi - lo))
    return out


def build_kernel(nc):
    x = nc.dram_tensor("x", [S, N, N], F32, kind="ExternalInput").ap()
    f = nc.dram_tensor("f", [S, N, N], F32, kind="ExternalInput").ap()
    ka = nc.dram_tensor("kernelA", [S, 9], F32, kind="ExternalInput").ap()
    kaf = nc.dram_tensor("kernelA_flip", [S, 9], F32,
                         kind="ExternalInput").ap()
    fc_w1 = [nc.dram_tensor(f"fc{i}_w1", [100, 9], F32, kind="ExternalInput").ap()
             for i in (1, 2)]
    fc_b1 = [nc.dram_tensor(f"fc{i}_b1", [100], F32, kind="ExternalInput").ap()
             for i in (1, 2)]
    fc_w2 = [nc.dram_tensor(f"fc{i}_w2", [147, 100], F32, kind="ExternalInput").ap()
             for i in (1, 2)]
    fc_b2 = [nc.dram_tensor(f"fc{i}_b2", [147], F32, kind="ExternalInput").ap()
             for i in (1, 2)]
    out = nc.dram_tensor("out", [S, N, N], F32, kind="ExternalOutput").ap()

    with TileContext(nc) as tc:
        with (
            tc.tile_pool(name="dram", bufs=1, space="DRAM") as dpool,
            tc.tile_pool(name="const", bufs=1) as cpool,
            tc.tile_pool(name="mlp", bufs=1) as mpool,
            tc.tile_pool(name="bands", bufs=2) as bpool,
            tc.tile_pool(name="banda", bufs=2) as bapool,
            tc.tile_pool(name="rt", bufs=2) as rtpool,
            tc.tile_pool(name="tp", bufs=2) as tppool,
            tc.tile_pool(name="xa", bufs=2) as xa_pool,
            tc.tile_pool(name="fr", bufs=3) as fr_pool,
            tc.tile_pool(name="stout", bufs=3) as st_pool,
            tc.tile_pool(name="tm3", bufs=4) as tm3_pool,
            tc.tile_pool(name="psA", bufs=1, space="PSUM") as psA,
            tc.tile_pool(name="ps1", bufs=4, space="PSUM") as ps1,
            tc.tile_pool(name="ps2", bufs=2, space="PSUM") as ps2,
            tc.tile_pool(name="psx", bufs=1, space="PSUM") as psx,
        ):
            tablesA = dpool.tile([TBLA_TOTAL], F32)
            tablesW = dpool.tile([TBLW_TOTAL], F32)

            # ---- constants
            ident = cpool.tile([128, 128], F32)
            nc.gpsimd.memset(ident, 0.0)
            nc.gpsimd.affine_select(
                out=ident, in_=ident, compare_op=mybir.AluOpType.not_equal,
                fill=1.0, base=0, pattern=[[-1, 128]], channel_multiplier=1)
            # anti-diagonal reversal Rev[k,p] = d(k+p=127)
            rev_f = cpool.tile([128, 128], F32)
            nc.gpsimd.memset(rev_f, 0.0)
            nc.gpsimd.affine_select(
                out=rev_f, in_=rev_f, compare_op=mybir.AluOpType.not_equal,
                fill=1.0, base=-127, pattern=[[1, 128]], channel_multiplier=1)
            rev = cpool.tile([128, 128], F32R)
            nc.scalar.copy(rev, rev_f)   # round to f32r for the PE
            # double identity D[p, c] = d(p = c - 128), c in [128, 256)
            did_f = cpool.tile([128, 384], F32)
            nc.gpsimd.memset(did_f, 0.0)
            nc.gpsimd.affine_select(
                out=did_f, in_=did_f, compare_op=mybir.AluOpType.not_equal,
                fill=1.0, base=128, pattern=[[-1, 384]], channel_multiplier=1)
            d_full = cpool.tile([128, 384], F32R)
            nc.scalar.copy(d_full, did_f)
            d_clip = cpool.tile([128, 384], F32R)
            nc.scalar.copy(d_clip, did_f)
            # zero partitions 126,127 of the diagonal = zero cols 254,255
            nc.vector.memset(d_clip.bitcast(F32)[:, 254:256], 0.0)

            # ---- zero-fill tables (6120 = 8*765; 85680 = 112*765)
            ztA = tm3_pool.tile([8, 765], F32, name="ztA", tag="tm3")
            nc.vector.memset(ztA, 0.0)
            nc.gpsimd.dma_start(_sub_ap(tablesA, [[765, 8], [1, 765]], 0), ztA)
            ztW = tm3_pool.tile([112, 765], F32, name="ztW", tag="tm3")
            nc.vector.memset(ztW, 0.0)
            nc.gpsimd.dma_start(
                _sub_ap(tablesW, [[765, 112], [1, 765]], 0), ztW)

            # ---- A tables from host-flipped kernelA (no MLP dependency):
            # T_A[(s*3+kx)*255 + 125 + ky'] = A_flip[ky', kx]
            vTf = mpool.tile([9, S], F32, name="vTf")
            nc.sync.dma_start(vTf, kaf.rearrange("s k -> k s"))
            for kyf in range(3):
                nc.gpsimd.dma_start(
                    _sub_ap(tablesA, [[TBL, 3], [3 * TBL, S]], 125 + kyf),
                    vTf[3 * kyf:3 * kyf + 3, :])

            # ---------------- MLP + weight staging ----------------
            vT = mpool.tile([9, S], F32)
            nc.sync.dma_start(vT, ka.rearrange("s k -> k s"))

            w_sb = {}  # (layer i, map m) -> [49, S] conv weights
            for i in range(2):
                w1n = mpool.tile([100, 9], F32, name=f"w1n{i}")
                nc.sync.dma_start(w1n, fc_w1[i])
                W1T = mpool.tile([9, 100], F32, name=f"W1T{i}")
                t1 = psx.tile([9, 100], F32, name=f"t1_{i}", tag="aux")
                nc.tensor.transpose(t1, w1n, ident[:100, :100])
                nc.vector.tensor_copy(W1T, t1)

                b1 = mpool.tile([100, 1], F32, name=f"b1_{i}")
                nc.sync.dma_start(b1, fc_b1[i].unsqueeze(1))

                w2n_a = mpool.tile([128, 100], F32, name=f"w2na{i}")
                nc.sync.dma_start(w2n_a, fc_w2[i][0:128, :])
                w2n_b = mpool.tile([19, 100], F32, name=f"w2nb{i}")
                nc.sync.dma_start(w2n_b, fc_w2[i][128:147, :])
                W2T = mpool.tile([100, 147], F32, name=f"W2T{i}")
                tr_a = psx.tile([100, 128], F32, name=f"tra{i}", tag="aux")
                nc.tensor.transpose(tr_a, w2n_a, ident)
                nc.vector.tensor_copy(W2T[:, 0:128], tr_a)
                tr_b = psx.tile([100, 19], F32, name=f"trb{i}", tag="aux")
                nc.tensor.transpose(tr_b, w2n_b, ident[:19, :19])
                nc.vector.tensor_copy(W2T[:, 128:147], tr_b)

                h_pre = psx.tile([100, S], F32, name=f"hpre{i}", tag="aux")
                nc.tensor.matmul(h_pre, W1T, vT, start=True, stop=True)
                h = mpool.tile([100, S], F32, name=f"h{i}")
                nc.scalar.activation(
                    h, h_pre, mybir.ActivationFunctionType.Gelu, bias=b1)

                for m in range(ML):
                    b2m = mpool.tile([49, 1], F32, name=f"b2_{i}_{m}")
                    nc.sync.dma_start(
                        b2m, fc_b2[i][49 * m:49 * m + 49].unsqueeze(1))
                    wp = psx.tile([49, S], F32, name=f"wp{i}{m}", tag="aux")
                    nc.tensor.matmul(wp, W2T[:, 49 * m:49 * m + 49], h,
                                     start=True, stop=True)
                    wsb = mpool.tile([49, S], F32, name=f"w_{i}_{m}")
                    nc.scalar.activation(
                        wsb, wp, mybir.ActivationFunctionType.Identity,
                        bias=b2m)
                    w_sb[(i, m)] = wsb

            # ---- batched scatters (ascending everywhere; fc2 rows are
            # host-flipped so stage-2 uses the same +ky layout):
            # T_W[(s*42 + base + m*7 + kx)*255 + 121 + ky] = w[ky, kx]
            for i, base in ((0, SLOT_S1), (1, SLOT_S2)):
                for m in range(ML):
                    for ky in range(KK):
                        nc.gpsimd.dma_start(
                            _sub_ap(tablesW,
                                    [[TBL, KK], [NSLOTW * TBL, S]],
                                    (base + m * KK) * TBL + 121 + ky),
                            w_sb[(i, m)][KK * ky:KK * ky + KK, :])

            # ---------------- main per-sample loop (software-pipelined) ----
            bb_t, ba_t, rt_t, tp_t, xt_t = {}, {}, {}, {}, {}

            def band(s, slot, m0, m1):
                b = bb_t[s]
                return b[:, slot * BANDW + m0:slot * BANDW + m1]

            def emit_band_load(s):
                baw = bapool.tile([128, 3 * BANDW], F32R, name=f"baw{s}",
                                  tag="baw")
                nc.sync.dma_start(
                    _sub_ap(baw, [[3 * BANDW, 128], [BANDW, 3], [1, BANDW]],
                            0),
                    _sub_ap(tablesA.bitcast(F32R),
                            [[1, 128], [TBL, 3], [1, BANDW]], s * 3 * TBL))
                ba_t[s] = [baw]      # [0]=window, rev appends reversed
                bb = bpool.tile([128, BFW], F32R, name=f"bb{s}", tag="bb")
                bb_t[s] = bb
                nc.sync.dma_start(
                    _sub_ap(bb, [[BFW, 128], [BANDW, 21], [1, BANDW]], 0),
                    _sub_ap(tablesW.bitcast(F32R),
                            [[1, 128], [TBL, 21], [1, BANDW]],
                            s * NSLOTW * TBL))
                nc.scalar.dma_start(
                    _sub_ap(bb, [[BFW, 128], [BANDW, 21], [1, BANDW]],
                            21 * BANDW),
                    _sub_ap(tablesW.bitcast(F32R),
                            [[1, 128], [TBL, 21], [1, BANDW]],
                            (s * NSLOTW + 21) * TBL))

            def emit_x_load(s):
                xts = []
                for j, (o0, M) in enumerate(AX_CH):
                    row_start = o0 - 1
                    xt = xa_pool.tile([128, N + 2], F32R,
                                      name=f"xt{s}_{j}", tag=f"xa{j}")
                    xts.append(xt)
                    # pads are identical every sample; data DMAs write only
                    # the interior, so memset pads only on first buffer use.
                    if row_start + 128 > N:          # bottom: ones pad
                        nd = N - row_start
                        if s < 2:
                            nc.gpsimd.memset(xt.bitcast(F32), 1.0)
                            nc.gpsimd.memset(
                                xt[0:nd, 0:1].bitcast(F32), 0.0)
                        nc.sync.dma_start(
                            xt[0:nd, 1:N + 1],
                            x.bitcast(F32R)[s, row_start:N, :])
                    else:
                        lo = max(0, row_start)
                        p0 = lo - row_start
                        if s < 2:
                            if p0 > 0:
                                nc.gpsimd.memset(
                                    xt[0:p0, :].bitcast(F32), 0.0)
                            nc.gpsimd.memset(xt[:, 0:1].bitcast(F32), 0.0)
                            nc.gpsimd.memset(
                                xt[:, N + 1:N + 2].bitcast(F32), 1.0)
                        nc.sync.dma_start(
                            xt[p0:128, 1:N + 1],
                            x.bitcast(F32R)[s, lo:row_start + 128, :])
                xt_t[s] = xts

            def emit_rev(s):
                baw = ba_t[s][0]
                ba = bapool.tile([128, 3 * BANDW], F32R, name=f"ba{s}",
                                 tag="ba")
                pr = psx.tile([128, 3 * BANDW], F32, name=f"pr{s}", tag="aux")
                nc.tensor.matmul(pr, rev, baw, start=True, stop=True)
                nc.scalar.copy(ba, pr)
                ba_t[s].append(ba)

            def emit_rhs_tiles(s):
                rt, tp = [], []
                for k in range(5):
                    t = rtpool.tile([128, N + 6], F32R, name=f"rt{s}_{k}",
                                    tag=f"rt{k}")
                    rt.append(t)
                    if s < 2:
                        nc.gpsimd.memset(t.bitcast(F32), 0.0)
                    t2 = tppool.tile([128, ML * (N + 6)], F32R,
                                     name=f"tp{s}_{k}", tag=f"tp{k}")
                    tp.append(t2)
                    if s < 2:
                        nc.gpsimd.memset(t2.bitcast(F32), 0.0)
                rt_t[s], tp_t[s] = rt, tp

            def emit_ax(s):
                ba, rt = ba_t[s][1], rt_t[s]
                for j, (o0, M) in enumerate(AX_CH):
                    xt = xt_t[s][j]
                    ft = fr_pool.tile([126, N], F32, name=f"ft{s}_{j}",
                                      tag="f")
                    nc.scalar.dma_start(ft[:M, :], f[s, o0:o0 + M, :])
                    ps = psA.tile([M, N], F32, name=f"psA{s}_{j}", tag="ax")
                    for kx in range(3):
                        nc.tensor.matmul(ps, ba[:, kx * BANDW:kx * BANDW + M],
                                         xt[:, kx:kx + N],
                                         start=(kx == 0), stop=(kx == 2))
                    rf = fr_pool.tile([126, N], F32, name=f"rf{s}_{j}",
                                      tag="rf")
                    nc.vector.tensor_sub(rf[:M, :], ft[:M, :], ps[:M, :])
                    for (k, p0, q0, n) in _overlaps_asc(o0, M):
                        eng = nc.sync if j % 2 == 0 else nc.scalar
                        eng.dma_start(
                            _sub_ap(rt[k], [[N + 6, n], [1, N]],
                                    p0 * (N + 6) + 3),
                            _sub_ap(rf.bitcast(F32R), [[N, n], [1, N]],
                                    q0 * N))

            def emit_stage1(s):
                rt, tp = rt_t[s], tp_t[s]
                for j, (o0, M) in enumerate(S7_CH):
                    qoff = 98 if M < 122 else 0   # lhsT col slice for c4
                    tm3 = tm3_pool.tile([122, ML * N], F32,
                                        name=f"tm3_{s}_{j}", tag="tm3")
                    for m in range(ML):
                        ps_ = ps1.tile([122, N], F32, name=f"ps1_{s}_{j}_{m}",
                                       tag="s1")
                        for kx in range(KK):
                            nc.tensor.matmul(
                                ps_[:M, :],
                                band(s, SLOT_S1 + m * KK + kx, qoff,
                                     qoff + M),
                                rt[j][:, kx:kx + N],
                                start=(kx == 0), stop=(kx == KK - 1))
                        nc.vector.tensor_copy(
                            tm3[:M, m * N:(m + 1) * N], ps_[:M, :])
                    for (k, p0, q0, n) in _overlaps_desc(o0, M, qoff):
                        eng = nc.sync if j % 2 == 0 else nc.scalar
                        eng.dma_start(
                            _sub_ap(tp[k],
                                    [[ML * (N + 6), n], [N + 6, ML], [1, N]],
                                    p0 * ML * (N + 6) + 3),
                            _sub_ap(tm3.bitcast(F32R),
                                    [[ML * N, n], [N, ML], [1, N]],
                                    q0 * ML * N))

            def emit_stage2(s):
                tp, xts = tp_t[s], xt_t[s]
                for j, (o0, M) in enumerate(S7_CH):
                    pg = ps2.tile([122, N], F32, name=f"ps2_{s}_{j}", tag="s2")
                    nmm = 21 + len(ADD_PAIRS[j])
                    idx = 0
                    for m in range(ML):
                        for kx in range(KK):
                            nc.tensor.matmul(
                                pg[:M, :],
                                band(s, SLOT_S2 + m * KK + kx, 0, M),
                                tp[j][:, m * (N + 6) + kx:
                                      m * (N + 6) + kx + N],
                                start=(idx == 0), stop=(idx == nmm - 1))
                            idx += 1
                    # out = x + G2 on the PE via double-identity bands
                    for (jx, shift, clip) in ADD_PAIRS[j]:
                        dd = d_clip if clip else d_full
                        nc.tensor.matmul(
                            pg[:M, :], dd[:, 128 + shift:128 + shift + M],
                            xts[jx][:, 1:N + 1],
                            start=False, stop=(idx == nmm - 1))
                        idx += 1
                    ob = st_pool.tile([122, N], F32, name=f"ob{s}_{j}",
                                      tag="ob")
                    nc.vector.tensor_copy(ob[:M, :], pg[:M, :])
                    nc.scalar.dma_start(out[s, o0:o0 + M, :], ob[:M, :])
                del bb_t[s], ba_t[s], rt_t[s], tp_t[s], xt_t[s]

            # prologue
            emit_band_load(0)
            emit_x_load(0)
            for s in range(S):
                emit_rev(s)
                emit_rhs_tiles(s)
                emit_ax(s)
                # prefetch next iteration inputs
                if s + 1 < S:
                    emit_band_load(s + 1)
                    emit_x_load(s + 1)
                if s >= 1:
                    emit_stage2(s - 1)
                emit_stage1(s)
            emit_stage2(S - 1)
    return nc


_CACHED = None


def _get_nc():
    global _CACHED
    if _CACHED is None:
        nc = bacc.Bacc("TRN2", debug=False, enable_asserts=False,
                       num_devices=NCORES)
        build_kernel(nc)
        nc.compile()
        _CACHED = nc
    return _CACHED


def make_in_maps(x, f, kernelA, fc1_w1, fc1_b1, fc1_w2, fc1_b2,
                 fc2_w1, fc2_b1, fc2_w2, fc2_b2):
    # stage-2 ky flip lives host-side: reorder fc2_w2/fc2_b2 output rows
    # (147 = 3 maps x 7 ky x 7 kx) so the on-device scatter is ascending.
    w2f = np.ascontiguousarray(
        np.asarray(fc2_w2, np.float32).reshape(ML, KK, KK, 100)[:, ::-1]
        .reshape(ML * KK * KK, 100))
    b2f = np.ascontiguousarray(
        np.asarray(fc2_b2, np.float32).reshape(ML, KK, KK)[:, ::-1]
        .reshape(ML * KK * KK))
    shared = {
        "fc1_w1": np.ascontiguousarray(fc1_w1, np.float32),
        "fc1_b1": np.ascontiguousarray(fc1_b1, np.float32),
        "fc1_w2": np.ascontiguousarray(fc1_w2, np.float32),
        "fc1_b2": np.ascontiguousarray(fc1_b2, np.float32),
        "fc2_w1": np.ascontiguousarray(fc2_w1, np.float32),
        "fc2_b1": np.ascontiguousarray(fc2_b1, np.float32),
        "fc2_w2": w2f,
        "fc2_b2": b2f,
    }
    in_maps = []
    for c in range(NCORES):
        sl = slice(S * c, S * (c + 1))
        kac = np.ascontiguousarray(
            kernelA[sl, 0].reshape(S, 9), np.float32)
        kaflip = np.ascontiguousarray(
            kac.reshape(S, 3, 3)[:, ::-1].reshape(S, 9))
        in_maps.append({
            "x": np.ascontiguousarray(x[sl, 0], np.float32),
            "f": np.ascontiguousarray(f[sl, 0], np.float32),
            "kernelA": kac,
            "kernelA_flip": kaflip,
            **shared,
        })
    return in_maps


def kernel(x, f, kernelA, fc1_w1, fc1_b1, fc1_w2, fc1_b2,
           fc2_w1, fc2_b1, fc2_w2, fc2_b2):
    x = np.asarray(x)
    nc = _get_nc()
    in_maps = make_in_maps(x, f, kernelA, fc1_w1, fc1_b1, fc1_w2, fc1_b2,
                           fc2_w1, fc2_b1, fc2_w2, fc2_b2)
    res = bass_utils.run_bass_kernel_spmd(
        nc, in_maps, core_ids=list(range(NCORES)))
    outs = [res.results[c]["out"] for c in range(NCORES)]
    full = np.concatenate(outs, axis=0).reshape(64, 1, N, N).astype(np.float32)
    return full
